# revision 1
# baseline (speedup 1.0000x reference)
"""Cross-attention (global, batch-flattened K/V) Trainium2 kernel.

Problem: emb [16, 4096, 64]; two cross-attention halves:
  out_l2u = cross(q=emb[:8],  kv=emb[8:])   -> rows 0..7
  out_u2l = cross(q=emb[8:],  kv=emb[:8])   -> rows 8..15
cross(): q/k/v proj (64->512), s = einsum('bnc,nd->bcd', q, kflat),
InstanceNorm over (CH, B*CH) plane per b, softmax over d, ctx = a @ vflat^T,
out = ctx @ Wout.

Sharding: 16 independent (cross, q-batch) instances, 2 per core.
Cores 0-3: q from lower half (kv = upper), cores 4-7: q from upper
(kv = lower), so each core needs k/v projections of one half only.
No collectives; weights replicated.

Per-core dataflow (all matmuls in float32r, 1 cycle/row on the PE):
  phase A: vT[b'] = (kv[b'] @ Wv)^T via PE -> DRAM scratch [8, 512, 4096]
  per instance:
    q = emb_q @ Wq resident in SBUF (lhsT layout via PE transposes)
    s[c, d] accumulated in PSUM over n; k-chunks projected on the fly
      (kf never touches DRAM); stats (sum, sumsq) fused on PSUM drain
    InstanceNorm + exp fused into one ACT pass (scale/bias per partition),
      row-sums via accum_out; softmax division deferred to ctx output
    aT via PE transposes
    ctxT[c, n] accumulated in PSUM over d, vT streamed from DRAM
    out = ctxT^T @ Wout via PE, DMA to output
"""

import numpy as np
import concourse.bass as bass
import concourse.mybir as mybir
import concourse.tile as tile
from concourse import bacc
from concourse.bass_utils import run_bass_kernel_spmd

dt = mybir.dt
AF = mybir.ActivationFunctionType
ALU = mybir.AluOpType

B = 8            # batches per half
N = 4096         # sequence length
C = 64           # embedding channels
CH = 512         # num_heads * C
NB = N // 128    # 32 n-blocks
NCH = N // 512   # 8 chunks of 512
CB = CH // 128   # 4 c-blocks
D = B * CH       # 4096 flattened kv dim
EPS = 1e-5
MM = dt.float32r  # matmul operand dtype
PLANE = float(CH * D)  # InstanceNorm plane size per instance

_nc = None


def _build():
    nc = bacc.Bacc("TRN2", target_bir_lowering=False, debug=False, num_devices=8)

    embq = nc.declare_dram_parameter("embq", [2, N, C], dt.float32, isOutput=False)
    embkv = nc.declare_dram_parameter("embkv", [B, N, C], dt.float32, isOutput=False)
    Wq_d = nc.declare_dram_parameter("Wq", [C, CH], dt.float32, isOutput=False)
    Wk_d = nc.declare_dram_parameter("Wk", [C, CH], dt.float32, isOutput=False)
    Wv_d = nc.declare_dram_parameter("Wv", [C, CH], dt.float32, isOutput=False)
    Wout_d = nc.declare_dram_parameter("Wout", [CH, C], dt.float32, isOutput=False)
    ident_d = nc.declare_dram_parameter("ident", [128, 128], dt.float32, isOutput=False)
    ones_d = nc.declare_dram_parameter("ones", [128, 128], dt.float32, isOutput=False)
    out_d = nc.declare_dram_parameter("out", [2, C, N], dt.float32, isOutput=True)

    vT_dram = nc.dram_tensor("vT_scratch", [B, CH, N], MM)

    with tile.TileContext(nc) as tc:
        with (
            tc.tile_pool(name="const", bufs=1) as constp,
            tc.tile_pool(name="io", bufs=2) as iop,
            tc.tile_pool(name="embt", bufs=1) as embtp,
            tc.tile_pool(name="stream", bufs=4) as streamp,
            tc.tile_pool(name="big", bufs=2) as bigp,
            tc.tile_pool(name="small", bufs=2) as smallp,
            tc.tile_pool(name="ps", bufs=8, space="PSUM") as psp,
        ):
            # ---- constants ----
            ident = constp.tile([128, 128], dt.float32, tag="ident")
            nc.sync.dma_start(ident[:], ident_d[:])
            ones_f = iop.tile([128, 128], dt.float32, tag="wst")
            nc.sync.dma_start(ones_f[:], ones_d[:])
            ones_r = constp.tile([128, 128], MM, tag="ones_r")
            nc.vector.tensor_copy(out=ones_r[:], in_=ones_f[:])

            w_rs = {}
            for name, wd in (("Wq", Wq_d), ("Wk", Wk_d), ("Wv", Wv_d)):
                wst = iop.tile([C, CH], dt.float32, tag="wst")
                nc.sync.dma_start(wst[:], wd[:])
                wr = constp.tile([C, CH], MM, tag=f"{name}_r")
                nc.vector.tensor_copy(out=wr[:], in_=wst[:])
                w_rs[name] = wr
            Wq_r, Wk_r, Wv_r = w_rs["Wq"], w_rs["Wk"], w_rs["Wv"]

            wost = iop.tile([128, CB, C], dt.float32, tag="wst")
            nc.sync.dma_start(
                wost[:], Wout_d[:].rearrange("(cb p) c -> p cb c", p=128)
            )
            Wout_r = constp.tile([128, CB, C], MM, tag="Wout_r")
            nc.vector.tensor_copy(out=Wout_r[:], in_=wost[:])

            # ---- helper: build embT [64, N] (f32r) for one batch ----
            def build_embT(src):  # src: DRAM AP [N, C] fp32
                et = embtp.tile([C, N], MM, tag="embT")
                for h in range(2):
                    lt = iop.tile([128, NB // 2, C], dt.float32, tag="embload")
                    nc.sync.dma_start(
                        lt[:],
                        src[h * (N // 2):(h + 1) * (N // 2), :].rearrange(
                            "(nb p) c -> p nb c", p=128
                        ),
                    )
                    for g in range(4):  # 4 transpose groups of 4 n-blocks
                        pt = psp.tile([128, 512], dt.float32, tag="pp")
                        for j in range(4):
                            nc.tensor.transpose(
                                pt[0:C, j * 128:(j + 1) * 128],
                                lt[:, g * 4 + j, :],
                                ident[:],
                            )
                        base = (h * 16 + g * 4) * 128
                        nc.vector.tensor_copy(
                            out=et[:, base:base + 512], in_=pt[0:C, :]
                        )
                return et

            # ---- phase A: vT for all kv batches -> DRAM ----
            for b in range(B):
                et = build_embT(embkv[b])
                for cb in range(CB):
                    for g in range(NCH):
                        pt = psp.tile([128, 512], dt.float32, tag="pp")
                        nc.tensor.matmul(
                            pt[:],
                            Wv_r[:, cb * 128:(cb + 1) * 128],
                            et[:, g * 512:(g + 1) * 512],
                            start=True,
                            stop=True,
                        )
                        st = streamp.tile([128, 512], MM, tag="vst", bufs=2)
                        nc.vector.tensor_copy(out=st[:], in_=pt[:])
                        nc.sync.dma_start(
                            vT_dram[b, cb * 128:(cb + 1) * 128,
                                    g * 512:(g + 1) * 512],
                            st[:],
                        )

            # ---- per instance ----
            for inst in range(2):
                # q resident: [128, nb, ch] f32r
                et_q = build_embT(embq[inst])
                q_sb = bigp.tile([128, NB, CH], MM, tag="big")
                for nb in range(NB):
                    pt = psp.tile([128, 512], dt.float32, tag="pp")
                    nc.tensor.matmul(
                        pt[:],
                        et_q[:, nb * 128:(nb + 1) * 128],
                        Wq_r[:],
                        start=True,
                        stop=True,
                    )
                    nc.vector.tensor_copy(out=q_sb[:, nb, :], in_=pt[:])

                # s = q^T @ kflat, accumulated over n; k projected on the fly
                s_sb = bigp.tile([128, CB, N], dt.float32, tag="big")
                ssum = smallp.tile([128, CB, B], dt.float32, tag="ssum")
                ssq = smallp.tile([128, CB, B], dt.float32, tag="ssq")
                for db in range(B):
                    et = build_embT(embkv[db])
                    ps_s = [psp.tile([128, 512], dt.float32, tag="pp",
                                     name=f"ps_s{cb_}")
                            for cb_ in range(CB)]
                    for nb in range(NB):
                        ptk = psp.tile([128, 512], dt.float32, tag="pp")
                        nc.tensor.matmul(
                            ptk[:],
                            et[:, nb * 128:(nb + 1) * 128],
                            Wk_r[:],
                            start=True,
                            stop=True,
                        )
                        kf = streamp.tile([128, 512], MM, tag="kf")
                        nc.vector.tensor_copy(out=kf[:], in_=ptk[:])
                        for cb in range(CB):
                            nc.tensor.matmul(
                                ps_s[cb][:],
                                q_sb[:, nb, cb * 128:(cb + 1) * 128],
                                kf[:],
                                start=(nb == 0),
                                stop=(nb == NB - 1),
                            )
                    for cb in range(CB):
                        nc.scalar.activation(
                            s_sb[:, cb, db * 512:(db + 1) * 512],
                            ps_s[cb][:],
                            AF.Copy,
                            accum_out=ssum[:, cb, db:db + 1],
                        )
                        # Square in place on PSUM (after the copy has read it)
                        nc.scalar.activation(
                            ps_s[cb][:],
                            ps_s[cb][:],
                            AF.Square,
                            accum_out=ssq[:, cb, db:db + 1],
                        )

                # ---- InstanceNorm stats -> per-partition scale/bias ----
                red = smallp.tile([128, 2], dt.float32, tag="red")
                nc.vector.tensor_reduce(
                    out=red[:, 0:1], in_=ssum[:], axis=mybir.AxisListType.XY,
                    op=ALU.add,
                )
                nc.vector.tensor_reduce(
                    out=red[:, 1:2], in_=ssq[:], axis=mybir.AxisListType.XY,
                    op=ALU.add,
                )
                red_r = smallp.tile([128, 2], MM, tag="red_r")
                nc.vector.tensor_copy(out=red_r[:], in_=red[:])
                ptr = psp.tile([128, 512], dt.float32, tag="pp")
                # all-partition totals via ones matmul
                nc.tensor.matmul(
                    ptr[:, 0:2], ones_r[:], red_r[:], start=True, stop=True
                )
                stats = smallp.tile([128, 8], dt.float32, tag="stats")
                # mu = tot_sum / PLANE ; ex2 = tot_sq / PLANE
                nc.scalar.activation(
                    stats[:, 0:2], ptr[:, 0:2], AF.Copy, bias=0.0,
                    scale=1.0 / PLANE,
                )
                mu = stats[:, 0:1]
                ex2 = stats[:, 1:2]
                musq = stats[:, 2:3]
                var = stats[:, 3:4]
                std = stats[:, 4:5]
                rstd = stats[:, 5:6]
                nmr = stats[:, 6:7]
                nc.vector.tensor_tensor(out=musq, in0=mu, in1=mu, op=ALU.mult)
                nc.vector.tensor_tensor(out=var, in0=ex2, in1=musq,
                                        op=ALU.subtract)
                nc.vector.tensor_scalar_add(var, var, EPS)
                nc.scalar.activation(std, var, AF.Sqrt, bias=0.0)
                nc.vector.reciprocal(rstd, std)
                nc.vector.tensor_tensor(out=nmr, in0=mu, in1=rstd, op=ALU.mult)
                nc.scalar.mul(nmr, nmr, -1.0)

                # ---- softmax numerator: a = exp((s - mu) * rstd), in place ----
                den = smallp.tile([128, CB], dt.float32, tag="den")
                for cb in range(CB):
                    nc.scalar.activation(
                        s_sb[:, cb, :],
                        s_sb[:, cb, :],
                        AF.Exp,
                        bias=nmr,
                        scale=rstd,
                        accum_out=den[:, cb:cb + 1],
                    )
                inv_den = smallp.tile([128, CB], dt.float32, tag="invden")
                nc.vector.reciprocal(inv_den[:], den[:])

                # ---- aT via PE transposes ----
                aT = bigp.tile([128, NB, CH], MM, tag="big")
                for ds in range(NB):
                    for cb in range(CB):
                        pt = psp.tile([128, 512], dt.float32, tag="pp")
                        nc.tensor.transpose(
                            pt[:, 0:128],
                            s_sb[:, cb, ds * 128:(ds + 1) * 128],
                            ident[:],
                        )
                        nc.vector.tensor_copy(
                            out=aT[:, ds, cb * 128:(cb + 1) * 128],
                            in_=pt[:, 0:128],
                        )

                # ---- ctxT = (a @ vflat^T) / den ----
                ctxT = bigp.tile([128, CB, N], MM, tag="big")
                for g in range(NCH):
                    ps_c = [psp.tile([128, 512], dt.float32, tag="pp",
                                     name=f"ps_c{cb_}")
                            for cb_ in range(CB)]
                    for bq in range(B):  # 4 d-steps per kv batch slab
                        vf = streamp.tile([128, 4, 512], MM, tag="vf", bufs=2)
                        nc.sync.dma_start(
                            vf[:],
                            vT_dram[bq, :, g * 512:(g + 1) * 512].rearrange(
                                "(j p) n -> p j n", p=128
                            ),
                        )
                        for j in range(4):
                            ds = bq * 4 + j
                            for cb in range(CB):
                                nc.tensor.matmul(
                                    ps_c[cb][:],
                                    aT[:, ds, cb * 128:(cb + 1) * 128],
                                    vf[:, j, :],
                                    start=(ds == 0),
                                    stop=(ds == NB - 1),
                                )
                    for cb in range(CB):
                        nc.scalar.activation(
                            ctxT[:, cb, g * 512:(g + 1) * 512],
                            ps_c[cb][:],
                            AF.Copy,
                            scale=inv_den[:, cb:cb + 1],
                        )

                # ---- outT = Wout^T @ ctx^T  (out returned transposed; host
                # flips [C, N] -> [N, C]) ----
                for g in range(NCH):
                    po = psp.tile([128, 512], dt.float32, tag="pp")
                    for cb in range(CB):
                        nc.tensor.matmul(
                            po[0:C, :],
                            Wout_r[:, cb, :],
                            ctxT[:, cb, g * 512:(g + 1) * 512],
                            start=(cb == 0),
                            stop=(cb == CB - 1),
                        )
                    ot = streamp.tile([C, 512], dt.float32, tag="ot")
                    nc.vector.tensor_copy(out=ot[:], in_=po[0:C, :])
                    nc.sync.dma_start(
                        out_d[inst, :, g * 512:(g + 1) * 512], ot[:]
                    )

    nc.compile()
    return nc


def _get_nc():
    global _nc
    if _nc is None:
        _nc = _build()
    return _nc


def kernel(emb, Wq, Wk, Wv, Wout):
    emb = np.ascontiguousarray(emb, dtype=np.float32)
    Wq = np.ascontiguousarray(Wq, dtype=np.float32)
    Wk = np.ascontiguousarray(Wk, dtype=np.float32)
    Wv = np.ascontiguousarray(Wv, dtype=np.float32)
    Wout = np.ascontiguousarray(Wout, dtype=np.float32)
    emb_l, emb_u = emb[:B], emb[B:]
    ident = np.eye(128, dtype=np.float32)
    ones = np.ones((128, 128), dtype=np.float32)

    in_maps = []
    for core in range(8):
        if core < 4:
            qb, kvb = emb_l[2 * core:2 * core + 2], emb_u
        else:
            j = core - 4
            qb, kvb = emb_u[2 * j:2 * j + 2], emb_l
        in_maps.append({
            "embq": np.ascontiguousarray(qb), "embkv": np.ascontiguousarray(kvb),
            "Wq": Wq, "Wk": Wk, "Wv": Wv, "Wout": Wout, "ident": ident,
            "ones": ones,
        })

    res = run_bass_kernel_spmd(_get_nc(), in_maps, list(range(8))).results

    out = np.empty((2 * B, N, C), np.float32)
    for core in range(8):
        o = res[core]["out"].transpose(0, 2, 1)  # [2, C, N] -> [2, N, C]
        if core < 4:
            out[2 * core:2 * core + 2] = o
        else:
            j = core - 4
            out[B + 2 * j:B + 2 * j + 2] = o
    return out



# revision 8
# speedup vs baseline: 1.4487x; 1.4487x over previous
"""Cross-attention (global, batch-flattened K/V) Trainium2 kernel, v2.

Problem: emb [16, 4096, 64]; two cross-attention halves:
  out_l2u = cross(q=emb[:8],  kv=emb[8:])   -> rows 0..7
  out_u2l = cross(q=emb[8:],  kv=emb[:8])   -> rows 8..15
cross(): q/k/v proj (64->512), s = einsum('bnc,nd->bcd', q, kflat),
InstanceNorm over (CH, B*CH) plane per b, softmax over d, ctx = a @ vflat^T,
out = ctx @ Wout.

Sharding: 16 (cross, q-batch) instances, 2 per core. Cores 0-3: q from
lower half (kv = upper), cores 4-7: q from upper (kv = lower). Both
instances on a core share the same kv half. No collectives.

v2 design (vs v1): fp16 matmul operands everywhere (fp32 PSUM accum),
sT layout (s stored [d, c]: no aT transposes; softmax denominator via
ones-matmuls), embT of the kv half built once and kept resident, v
projected on the fly per n-chunk (no DRAM scratch at all), K=64
projections row-packed 2x via partition-offset tile concurrency,
instance-staggered emission so the PE stream stays dense (HAM warm).

Per-core dataflow:
  E: embT_kv [128, 4, 4096] (c + 64*(db%2) on partitions), embT_q likewise
  Q: q[inst] = embT_q.T @ Wq  (row-packed, both instances concurrent)
  S (per inst): for db: for nb: kf = emb_kv @ Wk (on the fly);
     sT[dblk] += kf[:,dblk].T @ q  (PSUM, 4 banks); drain -> sT fp16 +
     running sum/sumsq via ACT accum_out
  N/X/D (per inst): InstanceNorm stats -> exp in place -> den via
     ones-matmul accumulation; staggered so inst1's S covers inst0's N/X
  C: for g (n-chunks of 512): vT on the fly from embT_kv; per inst:
     ctxT[cb] += expT.T @ vT (32-step PSUM accum); drain * 1/den;
     out = Wout.T @ ctxT -> DMA
"""

import numpy as np
import concourse.bass as bass
import concourse.mybir as mybir
import concourse.tile as tile
from concourse import bacc
from concourse.bass_utils import run_bass_kernel_spmd

dt = mybir.dt
AF = mybir.ActivationFunctionType
ALU = mybir.AluOpType
F16 = dt.float16
F32 = dt.float32

B = 8            # batches per half
N = 4096         # sequence length
C = 64           # embedding channels
CH = 512         # num_heads * C
NB = N // 128    # 32 n-blocks
NG = N // 512    # 8 n-groups
CB = CH // 128   # 4 c-blocks
D = B * CH       # 4096 flattened kv dim
DB32 = D // 128  # 32 d-blocks
EPS = 1e-5
PLANE = float(CH * D)

_nc = None


def _build():
    nc = bacc.Bacc("TRN2", target_bir_lowering=False, debug=False, num_devices=8)

    embq = nc.declare_dram_parameter("embq", [2, N, C], F16, isOutput=False)
    embkv = nc.declare_dram_parameter("embkv", [B, N, C], F16, isOutput=False)
    # weights pre-replicated on rows (64 -> 128) host-side for row packing
    Wq_d = nc.declare_dram_parameter("Wq", [128, CH], F16, isOutput=False)
    Wk_d = nc.declare_dram_parameter("Wk", [128, CH], F16, isOutput=False)
    Wv_d = nc.declare_dram_parameter("Wv", [128, CH], F16, isOutput=False)
    # Wout rearranged host-side: [p, cb, c] = Wout[cb*128+p, c]
    Wout_d = nc.declare_dram_parameter("Wout", [128, CB, C], F16, isOutput=False)
    ident_d = nc.declare_dram_parameter("ident", [128, 128], F16, isOutput=False)
    ones_d = nc.declare_dram_parameter("ones", [128, 128], F32, isOutput=False)
    out_d = nc.declare_dram_parameter("out", [2, C, N], F32, isOutput=True)

    with tile.TileContext(nc) as tc:
        with (
            tc.tile_pool(name="const", bufs=1) as constp,
            tc.tile_pool(name="io", bufs=2) as iop,
            tc.tile_pool(name="lt", bufs=3) as ltp,
            tc.tile_pool(name="embt", bufs=1) as embtp,
            tc.tile_pool(name="big", bufs=2) as bigp,
            tc.tile_pool(name="sT", bufs=1) as sTp,
            tc.tile_pool(name="kf", bufs=4) as kfp,
            tc.tile_pool(name="ctxs", bufs=2) as ctxp,
            tc.tile_pool(name="ot", bufs=3) as otp,
            tc.tile_pool(name="small", bufs=1) as smallp,
            tc.tile_pool(name="ps", bufs=1, space="PSUM") as psp,
        ):
            # ---- constants (all fp16 direct, no conversion) ----
            ident = constp.tile([128, 128], F16, tag="ident")
            nc.sync.dma_start(ident[:], ident_d[:])
            Wq_s = constp.tile([128, CH], F16, tag="Wq")
            nc.sync.dma_start(Wq_s[:], Wq_d[:])
            Wk_s = constp.tile([128, CH], F16, tag="Wk")
            nc.sync.dma_start(Wk_s[:], Wk_d[:])
            Wv_s = constp.tile([128, CH], F16, tag="Wv")
            nc.sync.dma_start(Wv_s[:], Wv_d[:])
            Wout_s = constp.tile([128, CB, C], F16, tag="Wout")
            nc.sync.dma_start(Wout_s[:], Wout_d[:])
            ones_f = iop.tile([128, 128], F32, tag="ones_f")
            nc.sync.dma_start(ones_f[:], ones_d[:])
            ones_r = constp.tile([128, 128], dt.float32r, tag="ones_r")
            nc.vector.tensor_copy(out=ones_r[:], in_=ones_f[:])
            ones16 = constp.tile([128, 1], F16, tag="ones16")
            nc.vector.tensor_copy(out=ones16[:], in_=ones_f[:, 0:1])

            # ---- phase E: build embT tensors ----
            # embT_kv[c + 64*(db%2), db//2, n] ; embT_q[c + 64*inst, n]
            embT_kv = embtp.tile([128, B // 2, N], F16, tag="embT_kv")
            embT_q = embtp.tile([128, N], F16, tag="embT_q")

            def build_pair(src_e, src_o, dst):  # dst: AP [128, N]
                lt_e = ltp.tile([128, NB, C], F16, tag="lt")
                nc.sync.dma_start(
                    lt_e[:], src_e.rearrange("(nb p) c -> p nb c", p=128))
                lt_o = ltp.tile([128, NB, C], F16, tag="lt")
                nc.sync.dma_start(
                    lt_o[:], src_o.rearrange("(nb p) c -> p nb c", p=128))
                for G in range(NG):
                    pt = psp.tile([128, 512], F16, tag="pp", bufs=3)
                    for j in range(4):
                        nb = G * 4 + j
                        nc.tensor.transpose(
                            pt[0:C, j * 128:(j + 1) * 128],
                            lt_e[:, nb, :], ident[:])
                        nc.tensor.transpose(
                            pt[C:128, j * 128:(j + 1) * 128],
                            lt_o[:, nb, :], ident[:])
                    nc.vector.tensor_copy(
                        out=dst[:, G * 512:(G + 1) * 512], in_=pt[:])

            for p in range(B // 2):
                build_pair(embkv[2 * p], embkv[2 * p + 1], embT_kv[:, p, :])
            build_pair(embq[0], embq[1], embT_q[:])

            # ---- phase Q: q projections (both instances row-packed) ----
            q_sb = [bigp.tile([128, NB, CH], F16, tag="big", name=f"q{i}")
                    for i in range(2)]
            for nb in range(NB):
                for i in range(2):
                    r = 64 * i
                    q_ps = psp.tile([128, 512], F32, tag="pp", bufs=3)
                    nc.tensor.matmul(
                        q_ps[:],
                        embT_q[r:r + 64, nb * 128:(nb + 1) * 128],
                        Wq_s[r:r + 64, :],
                        start=True, stop=True)
                    nc.vector.tensor_copy(out=q_sb[i][:, nb, :], in_=q_ps[:])

            sT = [sTp.tile([128, DB32, CH], F16, tag=f"sT{i}", name=f"sT{i}")
                  for i in range(2)]
            ssum = [smallp.tile([128, DB32], F32, tag=f"ssum{i}", name=f"ssum{i}")
                    for i in range(2)]
            ssq = [smallp.tile([128, DB32], F32, tag=f"ssq{i}", name=f"ssq{i}")
                   for i in range(2)]

            # ---- phase S: sT = kflat.T @ q per instance ----
            def s_phase(inst):
                for db in range(B):
                    r = 64 * (db % 2)
                    s_ps = [psp.tile([128, 512], F32, tag="sacc", bufs=5,
                                     name=f"sacc{inst}_{db}_{k}")
                            for k in range(4)]
                    for nb in range(NB):
                        kf_ps = psp.tile([128, 512], F32, tag="pp", bufs=3)
                        nc.tensor.matmul(
                            kf_ps[:],
                            embT_kv[r:r + 64, db // 2,
                                    nb * 128:(nb + 1) * 128],
                            Wk_s[r:r + 64, :],
                            start=True, stop=True)
                        kf = kfp.tile([128, 512], F16, tag="kf")
                        nc.vector.tensor_copy(out=kf[:], in_=kf_ps[:])
                        for k in range(4):
                            nc.tensor.matmul(
                                s_ps[k][:],
                                kf[:, k * 128:(k + 1) * 128],
                                q_sb[inst][:, nb, :],
                                start=(nb == 0), stop=(nb == NB - 1))
                    for k in range(4):
                        dk = db * 4 + k
                        nc.scalar.activation(
                            sT[inst][:, dk, :], s_ps[k][:], AF.Copy,
                            accum_out=ssum[inst][:, dk:dk + 1])
                        nc.scalar.activation(
                            s_ps[k][:], s_ps[k][:], AF.Square,
                            accum_out=ssq[inst][:, dk:dk + 1])

            # ---- stats + exp + den per instance ----
            stats = [smallp.tile([128, 8], F32, tag=f"stats{i}", name=f"stats{i}")
                     for i in range(2)]
            inv_den = [smallp.tile([128, CB], F32, tag=f"invden{i}", name=f"invden{i}")
                       for i in range(2)]

            def n_phase(inst):  # InstanceNorm stats
                red = smallp.tile([128, 2], F32, tag=f"red{inst}")
                nc.vector.tensor_reduce(
                    out=red[:, 0:1], in_=ssum[inst][:],
                    axis=mybir.AxisListType.X, op=ALU.add)
                nc.vector.tensor_reduce(
                    out=red[:, 1:2], in_=ssq[inst][:],
                    axis=mybir.AxisListType.X, op=ALU.add)
                red_r = smallp.tile([128, 2], dt.float32r, tag=f"redr{inst}")
                nc.vector.tensor_copy(out=red_r[:], in_=red[:])
                ptr = psp.tile([128, 512], F32, tag="pp", bufs=3)
                nc.tensor.matmul(
                    ptr[:, 0:2], ones_r[:], red_r[:], start=True, stop=True)
                st = stats[inst]
                nc.scalar.activation(
                    st[:, 0:2], ptr[:, 0:2], AF.Copy, bias=0.0,
                    scale=1.0 / PLANE)
                mu, ex2 = st[:, 0:1], st[:, 1:2]
                musq, var = st[:, 2:3], st[:, 3:4]
                std, rstd, nmr = st[:, 4:5], st[:, 5:6], st[:, 6:7]
                nc.vector.tensor_tensor(out=musq, in0=mu, in1=mu, op=ALU.mult)
                nc.vector.tensor_tensor(out=var, in0=ex2, in1=musq,
                                        op=ALU.subtract)
                nc.vector.tensor_scalar_add(var, var, EPS)
                nc.scalar.activation(std, var, AF.Sqrt, bias=0.0)
                nc.vector.reciprocal(rstd, std)
                nc.vector.tensor_tensor(out=nmr, in0=mu, in1=rstd,
                                        op=ALU.mult)
                nc.scalar.mul(nmr, nmr, -1.0)

            def x_phase(inst):  # exp in place, chunked
                st = stats[inst]
                for G in range(NG):
                    nc.scalar.activation(
                        sT[inst][:, G * 4:(G + 1) * 4, :],
                        sT[inst][:, G * 4:(G + 1) * 4, :],
                        AF.Exp, bias=st[:, 6:7], scale=st[:, 5:6])

            def d_phase(inst):  # softmax denominator via ones matmuls
                den_ps = [psp.tile([128, 512], F32, tag="sacc", bufs=5,
                                   name=f"den{inst}_{cb}")
                          for cb in range(CB)]
                for dk in range(DB32):
                    for cb in range(CB):
                        nc.tensor.matmul(
                            den_ps[cb][:, 0:1],
                            sT[inst][:, dk, cb * 128:(cb + 1) * 128],
                            ones16[:],
                            start=(dk == 0), stop=(dk == DB32 - 1))
                for cb in range(CB):
                    nc.vector.reciprocal(inv_den[inst][:, cb:cb + 1],
                                         den_ps[cb][:, 0:1])

            s_phase(0)
            n_phase(0)
            s_phase(1)      # covers inst0's stats/exp window with PE work
            x_phase(0)
            n_phase(1)
            d_phase(0)
            x_phase(1)
            d_phase(1)

            # ---- phase C: v on the fly + ctx + out ----
            for G in range(NG):
                va = bigp.tile([128, B, CB, 512], F16, tag="big",
                               name=f"va{G}")
                for db in range(B):
                    r = 64 * (db % 2)
                    for chb in range(CB):
                        v_ps = psp.tile([128, 512], F32, tag="pp", bufs=3)
                        nc.tensor.matmul(
                            v_ps[:],
                            Wv_s[r:r + 64, chb * 128:(chb + 1) * 128],
                            embT_kv[r:r + 64, db // 2,
                                    G * 512:(G + 1) * 512],
                            start=True, stop=True)
                        nc.scalar.activation(va[:, db, chb, :], v_ps[:],
                                             AF.Copy)
                for inst in range(2):
                    ctx_ps = [psp.tile([128, 512], F32, tag="sacc", bufs=5,
                                       name=f"ctx{G}_{inst}_{cb}")
                              for cb in range(CB)]
                    for db in range(B):
                        for chb in range(CB):
                            dk = db * 4 + chb
                            for cb in range(CB):
                                nc.tensor.matmul(
                                    ctx_ps[cb][:],
                                    sT[inst][:, dk, cb * 128:(cb + 1) * 128],
                                    va[:, db, chb, :],
                                    start=(dk == 0), stop=(dk == DB32 - 1))
                    ctxs = ctxp.tile([128, CB, 512], F16, tag="ctxs")
                    for cb in range(CB):
                        nc.scalar.activation(
                            ctxs[:, cb, :], ctx_ps[cb][:], AF.Copy,
                            scale=inv_den[inst][:, cb:cb + 1])
                    out_ps = psp.tile([128, 512], F32, tag="pp", bufs=3)
                    for cb in range(CB):
                        nc.tensor.matmul(
                            out_ps[0:C, :],
                            Wout_s[:, cb, :],
                            ctxs[:, cb, :],
                            start=(cb == 0), stop=(cb == CB - 1))
                    ot = otp.tile([C, 512], F32, tag="ot")
                    nc.vector.tensor_copy(out=ot[:], in_=out_ps[0:C, :])
                    nc.sync.dma_start(
                        out_d[inst, :, G * 512:(G + 1) * 512], ot[:])

    nc.compile()
    return nc


def _get_nc():
    global _nc
    if _nc is None:
        _nc = _build()
    return _nc


def make_in_maps(emb, Wq, Wk, Wv, Wout):
    """Per-core input dicts (8 cores). Host-side fp16 casts + replication."""
    emb16 = np.ascontiguousarray(emb, dtype=np.float16)
    Wq16 = np.concatenate([Wq, Wq], axis=0).astype(np.float16)
    Wk16 = np.concatenate([Wk, Wk], axis=0).astype(np.float16)
    Wv16 = np.concatenate([Wv, Wv], axis=0).astype(np.float16)
    Wout16 = np.ascontiguousarray(
        Wout.reshape(CB, 128, C).transpose(1, 0, 2)).astype(np.float16)
    ident = np.eye(128, dtype=np.float16)
    ones = np.ones((128, 128), np.float32)
    emb_l, emb_u = emb16[:B], emb16[B:]
    in_maps = []
    for core in range(8):
        if core < 4:
            qb, kvb = emb_l[2 * core:2 * core + 2], emb_u
        else:
            j = core - 4
            qb, kvb = emb_u[2 * j:2 * j + 2], emb_l
        in_maps.append({
            "embq": np.ascontiguousarray(qb),
            "embkv": np.ascontiguousarray(kvb),
            "Wq": Wq16, "Wk": Wk16, "Wv": Wv16, "Wout": Wout16,
            "ident": ident, "ones": ones,
        })
    return in_maps


def kernel(emb, Wq, Wk, Wv, Wout):
    in_maps = make_in_maps(np.asarray(emb), np.asarray(Wq), np.asarray(Wk),
                           np.asarray(Wv), np.asarray(Wout))
    res = run_bass_kernel_spmd(_get_nc(), in_maps, list(range(8))).results
    out = np.empty((2 * B, N, C), np.float32)
    for core in range(8):
        o = res[core]["out"].transpose(0, 2, 1)  # [2, C, N] -> [2, N, C]
        if core < 4:
            out[2 * core:2 * core + 2] = o
        else:
            j = core - 4
            out[B + 2 * j:B + 2 * j + 2] = o
    return out


# revision 15
# speedup vs baseline: 1.5475x; 1.0682x over previous
"""Cross-attention (global, batch-flattened K/V) Trainium2 kernel, v3.

Problem: emb [16, 4096, 64]; two cross-attention halves:
  out_l2u = cross(q=emb[:8],  kv=emb[8:])   -> rows 0..7
  out_u2l = cross(q=emb[8:],  kv=emb[:8])   -> rows 8..15
cross(): q/k/v proj (64->512), s = einsum('bnc,nd->bcd', q, kflat),
InstanceNorm over (CH, B*CH) plane per b, softmax over d, ctx = a @ vflat^T,
out = ctx @ Wout.

Sharding: 16 (cross, q-batch) instances, 2 per core. Cores 0-3: q from
lower half (kv = upper), cores 4-7: q from upper (kv = lower). Both
instances on a core share the same kv half. No collectives.

Design: fp16 matmul operands (fp32 PSUM accum), sT layout (s stored
[d, c]: no aT transposes; softmax denominator via ones-matmuls), embT
of the kv half resident (built with paired [128,128] transposes, two
batches' channels stacked on partitions), v projected on the fly per
n-chunk (no DRAM scratch), kf projection software-pipelined two steps
ahead of its consumers, emission interleaved (embT builds between
s-phase batches; den between ctx passes) to keep the PE dense and warm.

Per-core phases:
  Eq/Q: embT_q + q projections        S(i): sT = kflat.T @ q, per batch,
  Ekv p: embT_kv pair p                     kf projected on the fly
  N(i): InstanceNorm stats             X(i): exp in place (ACT)
  D(i): softmax denom (ones matmuls)   C: per n-chunk: v proj + ctx + out
"""

import numpy as np
import concourse.bass as bass
import concourse.mybir as mybir
import concourse.tile as tile
from concourse import bacc
from concourse.bass_utils import run_bass_kernel_spmd

dt = mybir.dt
AF = mybir.ActivationFunctionType
ALU = mybir.AluOpType
F16 = dt.float16
F32 = dt.float32

B = 8            # batches per half
N = 4096         # sequence length
C = 64           # embedding channels
CH = 512         # num_heads * C
NB = N // 128    # 32 n-blocks
NG = N // 512    # 8 n-groups
CB = CH // 128   # 4 c-blocks
D = B * CH       # 4096 flattened kv dim
DB32 = D // 128  # 32 d-blocks
EPS = 1e-5
PLANE = float(CH * D)

_nc = None


def _build():
    nc = bacc.Bacc("TRN2", target_bir_lowering=False, debug=False, num_devices=8)

    embq = nc.declare_dram_parameter("embq", [2, N, C], F16, isOutput=False)
    embkv = nc.declare_dram_parameter("embkv", [B, N, C], F16, isOutput=False)
    # weights pre-replicated on rows (64 -> 128) host-side for row packing
    Wq_d = nc.declare_dram_parameter("Wq", [128, CH], F16, isOutput=False)
    Wk_d = nc.declare_dram_parameter("Wk", [128, CH], F16, isOutput=False)
    Wv_d = nc.declare_dram_parameter("Wv", [128, CH], F16, isOutput=False)
    # Wout rearranged host-side: [p, cb, c] = Wout[cb*128+p, c]
    Wout_d = nc.declare_dram_parameter("Wout", [128, CB, C], F16, isOutput=False)
    ident_d = nc.declare_dram_parameter("ident", [128, 128], F16, isOutput=False)
    ones_d = nc.declare_dram_parameter("ones", [128, 128], F32, isOutput=False)
    out_d = nc.declare_dram_parameter("out", [2, C, N], F32, isOutput=True)

    with tile.TileContext(nc) as tc:
        with (
            tc.tile_pool(name="const", bufs=1) as constp,
            tc.tile_pool(name="io", bufs=2) as iop,
            tc.tile_pool(name="lt", bufs=2) as ltp,
            tc.tile_pool(name="embt", bufs=1) as embtp,
            tc.tile_pool(name="big", bufs=2) as bigp,
            tc.tile_pool(name="sT", bufs=1) as sTp,
            tc.tile_pool(name="kf", bufs=4) as kfp,
            tc.tile_pool(name="ctxs", bufs=2) as ctxp,
            tc.tile_pool(name="ot", bufs=2) as otp,
            tc.tile_pool(name="small", bufs=1) as smallp,
            tc.tile_pool(name="ps", bufs=1, space="PSUM") as psp,
        ):
            # ---- constants (all fp16 direct, no conversion) ----
            ident = constp.tile([128, 128], F16, tag="ident")
            nc.sync.dma_start(ident[:], ident_d[:])
            Wq_s = constp.tile([128, CH], F16, tag="Wq")
            nc.sync.dma_start(Wq_s[:], Wq_d[:])
            Wk_s = constp.tile([128, CH], F16, tag="Wk")
            nc.sync.dma_start(Wk_s[:], Wk_d[:])
            Wv_s = constp.tile([128, CH], F16, tag="Wv")
            nc.sync.dma_start(Wv_s[:], Wv_d[:])
            Wout_s = constp.tile([128, CB, C], F16, tag="Wout")
            nc.sync.dma_start(Wout_s[:], Wout_d[:])
            ones_f = iop.tile([128, 128], F32, tag="ones_f")
            nc.sync.dma_start(ones_f[:], ones_d[:])
            ones_r = constp.tile([128, 128], dt.float32r, tag="ones_r")
            nc.vector.tensor_copy(out=ones_r[:], in_=ones_f[:])
            ones16 = constp.tile([128, 1], F16, tag="ones16")
            nc.vector.tensor_copy(out=ones16[:], in_=ones_f[:, 0:1])

            # ---- embT layout ----
            # embT_kv[c + 64*(db%2), db//2, n] ; embT_q[c + 64*inst, n]
            embT_kv = embtp.tile([128, B // 2, N], F16, tag="embT_kv")
            embT_q = embtp.tile([128, N], F16, tag="embT_q")

            # prefetch all emb loads up front (pair-interleaved on channels:
            # lt[p, nb, b*64+c] = src[b, nb*128+p, c])
            def load_pair(src2, nm):
                lt = ltp.tile([128, NB, 2, C], F16, tag="lt", name=nm)
                for b in range(2):
                    nc.sync.dma_start(
                        lt[:, :, b, :],
                        src2[b].rearrange("(nb p) c -> p nb c", p=128))
                return lt

            lt_q = load_pair(embq[:], "ltq")
            lt_kv = [load_pair(embkv[2 * p:2 * p + 2], f"ltkv{p}")
                     for p in range(B // 2)]

            def build_pair(lt, dst):  # dst: AP [128, N]
                for G in range(NG):
                    pt = psp.tile([128, 512], F16, tag="pp", bufs=3)
                    for j in range(4):
                        nc.tensor.transpose(
                            pt[:, j * 128:(j + 1) * 128],
                            lt[:, G * 4 + j, :, :], ident[:])
                    nc.vector.tensor_copy(
                        out=dst[:, G * 512:(G + 1) * 512], in_=pt[:])

            # ---- embT_q + phase Q (warms the PE early with real matmuls) ----
            build_pair(lt_q, embT_q[:])
            q_sb = [bigp.tile([128, NB, CH], F16, tag="big", name=f"q{i}")
                    for i in range(2)]
            for nb in range(NB):
                for i in range(2):
                    r = 64 * i
                    q_ps = psp.tile([128, 512], F32, tag="pp", bufs=3)
                    nc.tensor.matmul(
                        q_ps[:],
                        embT_q[r:r + 64, nb * 128:(nb + 1) * 128],
                        Wq_s[r:r + 64, :],
                        start=True, stop=True)
                    nc.vector.tensor_copy(out=q_sb[i][:, nb, :], in_=q_ps[:])

            sT = [sTp.tile([128, DB32, CH], F16, tag=f"sT{i}", name=f"sT{i}")
                  for i in range(2)]
            ssum = [smallp.tile([128, DB32], F32, tag=f"ssum{i}",
                                name=f"ssum{i}") for i in range(2)]
            ssq = [smallp.tile([128, DB32], F32, tag=f"ssq{i}",
                               name=f"ssq{i}") for i in range(2)]
            sqscr = smallp.tile([128, 512], F16, tag="sqscr")

            # ---- phase S: sT = kflat.T @ q, kf pipelined 2 steps ahead ----
            def proj_kf(step):
                db, nb = divmod(step, NB)
                r = 64 * (db % 2)
                kf_ps = psp.tile([128, 512], F32, tag="pp", bufs=3)
                nc.tensor.matmul(
                    kf_ps[:],
                    embT_kv[r:r + 64, db // 2, nb * 128:(nb + 1) * 128],
                    Wk_s[r:r + 64, :],
                    start=True, stop=True)
                kf = kfp.tile([128, 512], F16, tag="kf")
                nc.vector.tensor_copy(out=kf[:], in_=kf_ps[:])
                return kf

            def s_phase(inst, dbs, pipe):
                # pipe: dict carrying the kf lookahead across calls
                for db in dbs:
                    s_ps = [psp.tile([128, 512], F32, tag="sacc", bufs=5,
                                     name=f"sacc{inst}_{db}_{k}")
                            for k in range(4)]
                    bound = (dbs[-1] + 1) * NB - 1
                    for nb in range(NB):
                        step = db * NB + nb
                        while pipe["next"] <= min(step + 2, bound):
                            pipe[pipe["next"]] = proj_kf(pipe["next"])
                            pipe["next"] += 1
                        kf = pipe.pop(step)
                        for k in range(4):
                            nc.tensor.matmul(
                                s_ps[k][:],
                                kf[:, k * 128:(k + 1) * 128],
                                q_sb[inst][:, nb, :],
                                start=(nb == 0), stop=(nb == NB - 1))
                    for k in range(4):
                        dk = db * 4 + k
                        nc.scalar.activation(
                            sT[inst][:, dk, :], s_ps[k][:], AF.Copy,
                            accum_out=ssum[inst][:, dk:dk + 1])
                        # sumsq from the fp16 copy (frees the PSUM bank early)
                        nc.scalar.activation(
                            sqscr[:], sT[inst][:, dk, :], AF.Square,
                            accum_out=ssq[inst][:, dk:dk + 1])

            # ---- stats / exp / den per instance ----
            stats = [smallp.tile([128, 8], F32, tag=f"stats{i}",
                                 name=f"stats{i}") for i in range(2)]
            inv_den = [smallp.tile([128, CB], F32, tag=f"invden{i}",
                                   name=f"invden{i}") for i in range(2)]

            def n_phase(inst):  # InstanceNorm stats
                red = smallp.tile([128, 2], F32, tag=f"red{inst}",
                                  name=f"red{inst}")
                nc.vector.tensor_reduce(
                    out=red[:, 0:1], in_=ssum[inst][:],
                    axis=mybir.AxisListType.X, op=ALU.add)
                nc.vector.tensor_reduce(
                    out=red[:, 1:2], in_=ssq[inst][:],
                    axis=mybir.AxisListType.X, op=ALU.add)
                red_r = smallp.tile([128, 2], dt.float32r, tag=f"redr{inst}",
                                    name=f"redr{inst}")
                nc.vector.tensor_copy(out=red_r[:], in_=red[:])
                ptr = psp.tile([128, 512], F32, tag="pp", bufs=3)
                nc.tensor.matmul(
                    ptr[:, 0:2], ones_r[:], red_r[:], start=True, stop=True)
                st = stats[inst]
                nc.scalar.activation(
                    st[:, 0:2], ptr[:, 0:2], AF.Copy, bias=0.0,
                    scale=1.0 / PLANE)
                mu, ex2 = st[:, 0:1], st[:, 1:2]
                musq, var = st[:, 2:3], st[:, 3:4]
                std, rstd, nmr = st[:, 4:5], st[:, 5:6], st[:, 6:7]
                nc.vector.tensor_tensor(out=musq, in0=mu, in1=mu, op=ALU.mult)
                nc.vector.tensor_tensor(out=var, in0=ex2, in1=musq,
                                        op=ALU.subtract)
                nc.vector.tensor_scalar_add(var, var, EPS)
                nc.scalar.activation(std, var, AF.Sqrt, bias=0.0)
                nc.vector.reciprocal(rstd, std)
                nc.vector.tensor_tensor(out=nmr, in0=mu, in1=rstd,
                                        op=ALU.mult)
                nc.scalar.mul(nmr, nmr, -1.0)

            def x_phase(inst):  # exp in place, chunked
                st = stats[inst]
                for G in range(NG):
                    nc.scalar.activation(
                        sT[inst][:, G * 4:(G + 1) * 4, :],
                        sT[inst][:, G * 4:(G + 1) * 4, :],
                        AF.Exp, bias=st[:, 6:7], scale=st[:, 5:6])

            def d_phase(inst):  # softmax denominator via ones matmuls
                den_ps = [psp.tile([128, 512], F32, tag="sacc", bufs=5,
                                   name=f"den{inst}_{cb}")
                          for cb in range(CB)]
                for dk in range(DB32):
                    for cb in range(CB):
                        nc.tensor.matmul(
                            den_ps[cb][:, 0:1],
                            sT[inst][:, dk, cb * 128:(cb + 1) * 128],
                            ones16[:],
                            start=(dk == 0), stop=(dk == DB32 - 1))
                for cb in range(CB):
                    nc.vector.reciprocal(inv_den[inst][:, cb:cb + 1],
                                         den_ps[cb][:, 0:1])

            # ---- emission: interleave embT_kv builds with s-phase(0) ----
            pipe = {"next": 0}
            for p in range(B // 2):
                build_pair(lt_kv[p], embT_kv[:, p, :])
                s_phase(0, [2 * p, 2 * p + 1], pipe)
            n_phase(0)
            pipe = {"next": 0}
            s_phase(1, list(range(B)), pipe)
            x_phase(0)
            d_phase(0)
            n_phase(1)
            x_phase(1)

            # ---- phase C: v on the fly + ctx + out; d_phase(1) spliced ----
            def ctx_pass(G, inst, va):
                ctx_ps = [psp.tile([128, 512], F32, tag="sacc", bufs=5,
                                   name=f"ctx{G}_{inst}_{cb}")
                          for cb in range(CB)]
                for db in range(B):
                    for chb in range(CB):
                        dk = db * 4 + chb
                        for cb in range(CB):
                            nc.tensor.matmul(
                                ctx_ps[cb][:],
                                sT[inst][:, dk, cb * 128:(cb + 1) * 128],
                                va[:, db, chb, :],
                                start=(dk == 0), stop=(dk == DB32 - 1))
                ctxs = ctxp.tile([128, CB, 512], F16, tag="ctxs")
                for cb in range(CB):
                    nc.scalar.activation(
                        ctxs[:, cb, :], ctx_ps[cb][:], AF.Copy,
                        scale=inv_den[inst][:, cb:cb + 1])
                out_ps = psp.tile([128, 512], F32, tag="pp", bufs=3)
                for cb in range(CB):
                    nc.tensor.matmul(
                        out_ps[0:C, :],
                        Wout_s[:, cb, :],
                        ctxs[:, cb, :],
                        start=(cb == 0), stop=(cb == CB - 1))
                ot = otp.tile([C, 512], F32, tag="ot")
                nc.vector.tensor_copy(out=ot[:], in_=out_ps[0:C, :])
                nc.sync.dma_start(
                    out_d[inst, :, G * 512:(G + 1) * 512], ot[:])

            for G in range(NG):
                va = bigp.tile([128, B, CB, 512], F16, tag="big",
                               name=f"va{G}")
                for db in range(B):
                    r = 64 * (db % 2)
                    for chb in range(CB):
                        v_ps = psp.tile([128, 512], F32, tag="pp", bufs=3)
                        nc.tensor.matmul(
                            v_ps[:],
                            Wv_s[r:r + 64, chb * 128:(chb + 1) * 128],
                            embT_kv[r:r + 64, db // 2,
                                    G * 512:(G + 1) * 512],
                            start=True, stop=True)
                        nc.scalar.activation(va[:, db, chb, :], v_ps[:],
                                             AF.Copy)
                ctx_pass(G, 0, va)
                if G == 0:
                    d_phase(1)  # PE work while ACT finishes exp(1)
                ctx_pass(G, 1, va)

    nc.compile()
    return nc


def _get_nc():
    global _nc
    if _nc is None:
        _nc = _build()
    return _nc


def make_in_maps(emb, Wq, Wk, Wv, Wout):
    """Per-core input dicts (8 cores). Host-side fp16 casts + replication."""
    emb16 = np.ascontiguousarray(emb, dtype=np.float16)
    Wq16 = np.concatenate([Wq, Wq], axis=0).astype(np.float16)
    Wk16 = np.concatenate([Wk, Wk], axis=0).astype(np.float16)
    Wv16 = np.concatenate([Wv, Wv], axis=0).astype(np.float16)
    Wout16 = np.ascontiguousarray(
        Wout.reshape(CB, 128, C).transpose(1, 0, 2)).astype(np.float16)
    ident = np.eye(128, dtype=np.float16)
    ones = np.ones((128, 128), np.float32)
    emb_l, emb_u = emb16[:B], emb16[B:]
    in_maps = []
    for core in range(8):
        if core < 4:
            qb, kvb = emb_l[2 * core:2 * core + 2], emb_u
        else:
            j = core - 4
            qb, kvb = emb_u[2 * j:2 * j + 2], emb_l
        in_maps.append({
            "embq": np.ascontiguousarray(qb),
            "embkv": np.ascontiguousarray(kvb),
            "Wq": Wq16, "Wk": Wk16, "Wv": Wv16, "Wout": Wout16,
            "ident": ident, "ones": ones,
        })
    return in_maps


def kernel(emb, Wq, Wk, Wv, Wout):
    in_maps = make_in_maps(np.asarray(emb), np.asarray(Wq), np.asarray(Wk),
                           np.asarray(Wv), np.asarray(Wout))
    res = run_bass_kernel_spmd(_get_nc(), in_maps, list(range(8))).results
    out = np.empty((2 * B, N, C), np.float32)
    for core in range(8):
        o = res[core]["out"].transpose(0, 2, 1)  # [2, C, N] -> [2, N, C]
        if core < 4:
            out[2 * core:2 * core + 2] = o
        else:
            j = core - 4
            out[B + 2 * j:B + 2 * j + 2] = o
    return out


# revision 18
# speedup vs baseline: 1.6032x; 1.0360x over previous
"""Cross-attention (global, batch-flattened K/V) Trainium2 kernel, v3.

Problem: emb [16, 4096, 64]; two cross-attention halves:
  out_l2u = cross(q=emb[:8],  kv=emb[8:])   -> rows 0..7
  out_u2l = cross(q=emb[8:],  kv=emb[:8])   -> rows 8..15
cross(): q/k/v proj (64->512), s = einsum('bnc,nd->bcd', q, kflat),
InstanceNorm over (CH, B*CH) plane per b, softmax over d, ctx = a @ vflat^T,
out = ctx @ Wout.

Sharding: 16 (cross, q-batch) instances, 2 per core. Cores 0-3: q from
lower half (kv = upper), cores 4-7: q from upper (kv = lower). Both
instances on a core share the same kv half. No collectives.

Design: fp16 matmul operands (fp32 PSUM accum), sT layout (s stored
[d, c]: no aT transposes; softmax denominator via ones-matmuls), embT
of the kv half resident (built with paired [128,128] transposes, two
batches' channels stacked on partitions), v projected on the fly per
n-chunk (no DRAM scratch), kf projection software-pipelined two steps
ahead of its consumers, emission interleaved (embT builds between
s-phase batches; den between ctx passes) to keep the PE dense and warm.

Per-core phases:
  Eq/Q: embT_q + q projections        S(i): sT = kflat.T @ q, per batch,
  Ekv p: embT_kv pair p                     kf projected on the fly
  N(i): InstanceNorm stats             X(i): exp in place (ACT)
  D(i): softmax denom (ones matmuls)   C: per n-chunk: v proj + ctx + out
"""

import numpy as np
import concourse.bass as bass
import concourse.mybir as mybir
import concourse.tile as tile
from concourse import bacc
from concourse.bass_utils import run_bass_kernel_spmd

dt = mybir.dt
AF = mybir.ActivationFunctionType
ALU = mybir.AluOpType
F16 = dt.float16
F32 = dt.float32

B = 8            # batches per half
N = 4096         # sequence length
C = 64           # embedding channels
CH = 512         # num_heads * C
NB = N // 128    # 32 n-blocks
NG = N // 512    # 8 n-groups
CB = CH // 128   # 4 c-blocks
D = B * CH       # 4096 flattened kv dim
DB32 = D // 128  # 32 d-blocks
EPS = 1e-5
PLANE = float(CH * D)

_nc = None


def _build():
    nc = bacc.Bacc("TRN2", target_bir_lowering=False, debug=False, num_devices=8)

    embq = nc.declare_dram_parameter("embq", [2, N, C], F16, isOutput=False)
    embkv = nc.declare_dram_parameter("embkv", [B, N, C], F16, isOutput=False)
    # weights pre-replicated on rows (64 -> 128) host-side for row packing
    Wq_d = nc.declare_dram_parameter("Wq", [128, CH], F16, isOutput=False)
    Wk_d = nc.declare_dram_parameter("Wk", [128, CH], F16, isOutput=False)
    Wv_d = nc.declare_dram_parameter("Wv", [128, CH], F16, isOutput=False)
    # Wout rearranged host-side: [p, cb, c] = Wout[cb*128+p, c]
    Wout_d = nc.declare_dram_parameter("Wout", [128, CB, C], F16, isOutput=False)
    ident_d = nc.declare_dram_parameter("ident", [128, 128], F16, isOutput=False)
    ones_d = nc.declare_dram_parameter("ones", [128, 128], F32, isOutput=False)
    out_d = nc.declare_dram_parameter("out", [2, C, N], F16, isOutput=True)

    with tile.TileContext(nc) as tc:
        with (
            tc.tile_pool(name="const", bufs=1) as constp,
            tc.tile_pool(name="io", bufs=2) as iop,
            tc.tile_pool(name="lt", bufs=2) as ltp,
            tc.tile_pool(name="embt", bufs=1) as embtp,
            tc.tile_pool(name="big", bufs=2) as bigp,
            tc.tile_pool(name="sT", bufs=1) as sTp,
            tc.tile_pool(name="kf", bufs=6) as kfp,
            tc.tile_pool(name="ctxs", bufs=1) as ctxp,
            tc.tile_pool(name="ot", bufs=2) as otp,
            tc.tile_pool(name="small", bufs=1) as smallp,
            tc.tile_pool(name="ps", bufs=1, space="PSUM") as psp,
        ):
            # ---- constants (all fp16 direct, no conversion) ----
            ident = constp.tile([128, 128], F16, tag="ident")
            nc.sync.dma_start(ident[:], ident_d[:])
            Wq_s = constp.tile([128, CH], F16, tag="Wq")
            nc.sync.dma_start(Wq_s[:], Wq_d[:])
            Wk_s = constp.tile([128, CH], F16, tag="Wk")
            nc.sync.dma_start(Wk_s[:], Wk_d[:])
            Wv_s = constp.tile([128, CH], F16, tag="Wv")
            nc.sync.dma_start(Wv_s[:], Wv_d[:])
            Wout_s = constp.tile([128, CB, C], F16, tag="Wout")
            nc.sync.dma_start(Wout_s[:], Wout_d[:])
            ones_f = iop.tile([128, 128], F32, tag="ones_f")
            nc.sync.dma_start(ones_f[:], ones_d[:])
            ones_r = constp.tile([128, 128], dt.float32r, tag="ones_r")
            nc.vector.tensor_copy(out=ones_r[:], in_=ones_f[:])
            ones16 = constp.tile([128, 1], F16, tag="ones16")
            nc.vector.tensor_copy(out=ones16[:], in_=ones_f[:, 0:1])

            # ---- embT layout ----
            # embT_kv[c + 64*(db%2), db//2, n] ; embT_q[c + 64*inst, n]
            embT_kv = embtp.tile([128, B // 2, N], F16, tag="embT_kv")
            embT_q = embtp.tile([128, N], F16, tag="embT_q")

            # prefetch all emb loads up front (pair-interleaved on channels:
            # lt[p, nb, b*64+c] = src[b, nb*128+p, c])
            def load_pair(src2, nm):
                lt = ltp.tile([128, NB, 2, C], F16, tag="lt", name=nm)
                for b in range(2):
                    nc.sync.dma_start(
                        lt[:, :, b, :],
                        src2[b].rearrange("(nb p) c -> p nb c", p=128))
                return lt

            lt_q = load_pair(embq[:], "ltq")
            lt_kv = [load_pair(embkv[2 * p:2 * p + 2], f"ltkv{p}")
                     for p in range(B // 2)]

            def build_pair(lt, dst):  # dst: AP [128, N]
                for G in range(NG):
                    pt = psp.tile([128, 512], F16, tag="pp", bufs=3)
                    for j in range(4):
                        nc.tensor.transpose(
                            pt[:, j * 128:(j + 1) * 128],
                            lt[:, G * 4 + j, :, :], ident[:])
                    nc.vector.tensor_copy(
                        out=dst[:, G * 512:(G + 1) * 512], in_=pt[:])

            # ---- embT_q + phase Q (warms the PE early with real matmuls) ----
            build_pair(lt_q, embT_q[:])
            q_sb = [bigp.tile([128, NB, CH], F16, tag="big", name=f"q{i}")
                    for i in range(2)]
            for nb in range(NB):
                for i in range(2):
                    r = 64 * i
                    q_ps = psp.tile([128, 512], F32, tag="pp", bufs=3)
                    nc.tensor.matmul(
                        q_ps[:],
                        embT_q[r:r + 64, nb * 128:(nb + 1) * 128],
                        Wq_s[r:r + 64, :],
                        start=True, stop=True)
                    nc.vector.tensor_copy(out=q_sb[i][:, nb, :], in_=q_ps[:])

            sT = [sTp.tile([128, DB32, CH], F16, tag=f"sT{i}", name=f"sT{i}")
                  for i in range(2)]
            ssum = [smallp.tile([128, DB32], F32, tag=f"ssum{i}",
                                name=f"ssum{i}") for i in range(2)]
            ssq = [smallp.tile([128, DB32], F32, tag=f"ssq{i}",
                               name=f"ssq{i}") for i in range(2)]
            sqscr = smallp.tile([128, 512], F16, tag="sqscr")

            # ---- phase S: sT = kflat.T @ q, kf pipelined 2 steps ahead ----
            def proj_kf(step):
                db, nb = divmod(step, NB)
                r = 64 * (db % 2)
                kf_ps = psp.tile([128, 512], F32, tag="pp", bufs=3)
                nc.tensor.matmul(
                    kf_ps[:],
                    embT_kv[r:r + 64, db // 2, nb * 128:(nb + 1) * 128],
                    Wk_s[r:r + 64, :],
                    start=True, stop=True)
                kf = kfp.tile([128, 512], F16, tag="kf")
                nc.vector.tensor_copy(out=kf[:], in_=kf_ps[:])
                return kf

            def s_phase(inst, dbs, pipe):
                # pipe: dict carrying the kf lookahead across calls
                for db in dbs:
                    s_ps = [psp.tile([128, 512], F32, tag="sacc", bufs=5,
                                     name=f"sacc{inst}_{db}_{k}")
                            for k in range(4)]
                    bound = (dbs[-1] + 1) * NB - 1
                    for nb in range(NB):
                        step = db * NB + nb
                        # project kf in bursts of 4 to amortize the K=64/128
                        # LDWEIGHTS row-group switch; stay 2-6 steps ahead
                        if pipe["next"] <= min(step + 2, bound):
                            hi = min(pipe["next"] + 3, bound)
                            while pipe["next"] <= hi:
                                pipe[pipe["next"]] = proj_kf(pipe["next"])
                                pipe["next"] += 1
                        kf = pipe.pop(step)
                        for k in range(4):
                            nc.tensor.matmul(
                                s_ps[k][:],
                                kf[:, k * 128:(k + 1) * 128],
                                q_sb[inst][:, nb, :],
                                start=(nb == 0), stop=(nb == NB - 1))
                    for k in range(4):
                        dk = db * 4 + k
                        nc.scalar.activation(
                            sT[inst][:, dk, :], s_ps[k][:], AF.Copy,
                            accum_out=ssum[inst][:, dk:dk + 1])
                        # sumsq from the fp16 copy (frees the PSUM bank early)
                        nc.scalar.activation(
                            sqscr[:], sT[inst][:, dk, :], AF.Square,
                            accum_out=ssq[inst][:, dk:dk + 1])

            # ---- stats / exp / den per instance ----
            stats = [smallp.tile([128, 8], F32, tag=f"stats{i}",
                                 name=f"stats{i}") for i in range(2)]
            inv_den = [smallp.tile([128, CB], F32, tag=f"invden{i}",
                                   name=f"invden{i}") for i in range(2)]

            def n_phase(inst):  # InstanceNorm stats
                red = smallp.tile([128, 2], F32, tag=f"red{inst}",
                                  name=f"red{inst}")
                nc.vector.tensor_reduce(
                    out=red[:, 0:1], in_=ssum[inst][:],
                    axis=mybir.AxisListType.X, op=ALU.add)
                nc.vector.tensor_reduce(
                    out=red[:, 1:2], in_=ssq[inst][:],
                    axis=mybir.AxisListType.X, op=ALU.add)
                red_r = smallp.tile([128, 2], dt.float32r, tag=f"redr{inst}",
                                    name=f"redr{inst}")
                nc.vector.tensor_copy(out=red_r[:], in_=red[:])
                ptr = psp.tile([128, 512], F32, tag="pp", bufs=3)
                nc.tensor.matmul(
                    ptr[:, 0:2], ones_r[:], red_r[:], start=True, stop=True)
                st = stats[inst]
                nc.scalar.activation(
                    st[:, 0:2], ptr[:, 0:2], AF.Copy, bias=0.0,
                    scale=1.0 / PLANE)
                mu, ex2 = st[:, 0:1], st[:, 1:2]
                musq, var = st[:, 2:3], st[:, 3:4]
                std, rstd, nmr = st[:, 4:5], st[:, 5:6], st[:, 6:7]
                nc.vector.tensor_tensor(out=musq, in0=mu, in1=mu, op=ALU.mult)
                nc.vector.tensor_tensor(out=var, in0=ex2, in1=musq,
                                        op=ALU.subtract)
                nc.vector.tensor_scalar_add(var, var, EPS)
                nc.scalar.activation(std, var, AF.Sqrt, bias=0.0)
                nc.vector.reciprocal(rstd, std)
                nc.vector.tensor_tensor(out=nmr, in0=mu, in1=rstd,
                                        op=ALU.mult)
                nc.scalar.mul(nmr, nmr, -1.0)

            def x_phase(inst):  # exp in place, chunked
                st = stats[inst]
                for G in range(NG):
                    nc.scalar.activation(
                        sT[inst][:, G * 4:(G + 1) * 4, :],
                        sT[inst][:, G * 4:(G + 1) * 4, :],
                        AF.Exp, bias=st[:, 6:7], scale=st[:, 5:6])

            def d_phase(inst):  # softmax denominator, dense 512-wide MMs
                den_ps = psp.tile([128, 512], F32, tag="pp", bufs=3)
                for dk in range(DB32):
                    nc.tensor.matmul(
                        den_ps[0:1, :], ones16[:], sT[inst][:, dk, :],
                        start=(dk == 0), stop=(dk == DB32 - 1))
                dr = smallp.tile([1, 512], F16, tag=f"denrow{inst}",
                                 name=f"denrow{inst}")
                nc.vector.tensor_copy(out=dr[:], in_=den_ps[0:1, :])
                # spread den[c] across partitions: K=1 matmuls per c-block
                spread = psp.tile([128, 512], F32, tag="pp", bufs=3)
                for cb in range(CB):
                    nc.tensor.matmul(
                        spread[:, cb:cb + 1],
                        dr[0:1, cb * 128:(cb + 1) * 128],
                        ones16[0:1, 0:1],
                        start=(cb == 0), stop=(cb == CB - 1))
                nc.vector.reciprocal(inv_den[inst][:], spread[:, 0:CB])

            # ---- emission: interleave embT_kv builds with s-phase(0) ----
            pipe = {"next": 0}
            for p in range(B // 2):
                build_pair(lt_kv[p], embT_kv[:, p, :])
                s_phase(0, [2 * p, 2 * p + 1], pipe)
            n_phase(0)
            pipe = {"next": 0}
            s_phase(1, list(range(B)), pipe)
            x_phase(0)
            d_phase(0)
            n_phase(1)
            x_phase(1)

            # ---- phase C: v on the fly + ctx + out; d_phase(1) spliced ----
            def ctx_pass(G, inst, va):
                ctx_ps = [psp.tile([128, 512], F32, tag="sacc", bufs=5,
                                   name=f"ctx{G}_{inst}_{cb}")
                          for cb in range(CB)]
                for db in range(B):
                    for chb in range(CB):
                        dk = db * 4 + chb
                        for cb in range(CB):
                            nc.tensor.matmul(
                                ctx_ps[cb][:],
                                sT[inst][:, dk, cb * 128:(cb + 1) * 128],
                                va[:, db, chb, :],
                                start=(dk == 0), stop=(dk == DB32 - 1))
                ctxs = ctxp.tile([128, CB, 512], F16, tag="ctxs")
                for cb in range(CB):
                    nc.scalar.activation(
                        ctxs[:, cb, :], ctx_ps[cb][:], AF.Copy,
                        scale=inv_den[inst][:, cb:cb + 1])
                out_ps = psp.tile([128, 512], F32, tag="pp", bufs=3)
                for cb in range(CB):
                    nc.tensor.matmul(
                        out_ps[0:C, :],
                        Wout_s[:, cb, :],
                        ctxs[:, cb, :],
                        start=(cb == 0), stop=(cb == CB - 1))
                ot = otp.tile([C, 512], F16, tag="ot")
                nc.vector.tensor_copy(out=ot[:], in_=out_ps[0:C, :])
                nc.sync.dma_start(
                    out_d[inst, :, G * 512:(G + 1) * 512], ot[:])

            for G in range(NG):
                va = bigp.tile([128, B, CB, 512], F16, tag="big",
                               name=f"va{G}")
                for db in range(B):
                    r = 64 * (db % 2)
                    for chb in range(CB):
                        v_ps = psp.tile([128, 512], F32, tag="pp", bufs=3)
                        nc.tensor.matmul(
                            v_ps[:],
                            Wv_s[r:r + 64, chb * 128:(chb + 1) * 128],
                            embT_kv[r:r + 64, db // 2,
                                    G * 512:(G + 1) * 512],
                            start=True, stop=True)
                        nc.scalar.activation(va[:, db, chb, :], v_ps[:],
                                             AF.Copy)
                ctx_pass(G, 0, va)
                if G == 0:
                    d_phase(1)  # PE work while ACT finishes exp(1)
                ctx_pass(G, 1, va)

    nc.compile()
    return nc


def _get_nc():
    global _nc
    if _nc is None:
        _nc = _build()
    return _nc


def make_in_maps(emb, Wq, Wk, Wv, Wout):
    """Per-core input dicts (8 cores). Host-side fp16 casts + replication."""
    emb16 = np.ascontiguousarray(emb, dtype=np.float16)
    Wq16 = np.concatenate([Wq, Wq], axis=0).astype(np.float16)
    Wk16 = np.concatenate([Wk, Wk], axis=0).astype(np.float16)
    Wv16 = np.concatenate([Wv, Wv], axis=0).astype(np.float16)
    Wout16 = np.ascontiguousarray(
        Wout.reshape(CB, 128, C).transpose(1, 0, 2)).astype(np.float16)
    ident = np.eye(128, dtype=np.float16)
    ones = np.ones((128, 128), np.float32)
    emb_l, emb_u = emb16[:B], emb16[B:]
    in_maps = []
    for core in range(8):
        if core < 4:
            qb, kvb = emb_l[2 * core:2 * core + 2], emb_u
        else:
            j = core - 4
            qb, kvb = emb_u[2 * j:2 * j + 2], emb_l
        in_maps.append({
            "embq": np.ascontiguousarray(qb),
            "embkv": np.ascontiguousarray(kvb),
            "Wq": Wq16, "Wk": Wk16, "Wv": Wv16, "Wout": Wout16,
            "ident": ident, "ones": ones,
        })
    return in_maps


def kernel(emb, Wq, Wk, Wv, Wout):
    in_maps = make_in_maps(np.asarray(emb), np.asarray(Wq), np.asarray(Wk),
                           np.asarray(Wv), np.asarray(Wout))
    res = run_bass_kernel_spmd(_get_nc(), in_maps, list(range(8))).results
    out = np.empty((2 * B, N, C), np.float32)
    for core in range(8):
        o = res[core]["out"].transpose(0, 2, 1)  # [2, C, N] -> [2, N, C]
        if core < 4:
            out[2 * core:2 * core + 2] = o
        else:
            j = core - 4
            out[B + 2 * j:B + 2 * j + 2] = o
    return out


# revision 20
# speedup vs baseline: 1.6156x; 1.0077x over previous
"""Cross-attention (global, batch-flattened K/V) Trainium2 kernel, v3.

Problem: emb [16, 4096, 64]; two cross-attention halves:
  out_l2u = cross(q=emb[:8],  kv=emb[8:])   -> rows 0..7
  out_u2l = cross(q=emb[8:],  kv=emb[:8])   -> rows 8..15
cross(): q/k/v proj (64->512), s = einsum('bnc,nd->bcd', q, kflat),
InstanceNorm over (CH, B*CH) plane per b, softmax over d, ctx = a @ vflat^T,
out = ctx @ Wout.

Sharding: 16 (cross, q-batch) instances, 2 per core. Cores 0-3: q from
lower half (kv = upper), cores 4-7: q from upper (kv = lower). Both
instances on a core share the same kv half. No collectives.

Design: fp16 matmul operands (fp32 PSUM accum), sT layout (s stored
[d, c]: no aT transposes; softmax denominator via ones-matmuls), embT
of the kv half resident (built with paired [128,128] transposes, two
batches' channels stacked on partitions), v projected on the fly per
n-chunk (no DRAM scratch), kf projection software-pipelined two steps
ahead of its consumers, emission interleaved (embT builds between
s-phase batches; den between ctx passes) to keep the PE dense and warm.

Per-core phases:
  Eq/Q: embT_q + q projections        S(i): sT = kflat.T @ q, per batch,
  Ekv p: embT_kv pair p                     kf projected on the fly
  N(i): InstanceNorm stats             X(i): exp in place (ACT)
  D(i): softmax denom (ones matmuls)   C: per n-chunk: v proj + ctx + out
"""

import numpy as np
import concourse.bass as bass
import concourse.mybir as mybir
import concourse.tile as tile
from concourse import bacc
from concourse.bass_utils import run_bass_kernel_spmd

dt = mybir.dt
AF = mybir.ActivationFunctionType
ALU = mybir.AluOpType
F16 = dt.float16
F32 = dt.float32

B = 8            # batches per half
N = 4096         # sequence length
C = 64           # embedding channels
CH = 512         # num_heads * C
NB = N // 128    # 32 n-blocks
NG = N // 512    # 8 n-groups
CB = CH // 128   # 4 c-blocks
D = B * CH       # 4096 flattened kv dim
DB32 = D // 128  # 32 d-blocks
EPS = 1e-5
PLANE = float(CH * D)

_nc = None


def _build():
    nc = bacc.Bacc("TRN2", target_bir_lowering=False, debug=False, num_devices=8)

    embq = nc.declare_dram_parameter("embq", [2, N, C], F16, isOutput=False)
    embkv = nc.declare_dram_parameter("embkv", [B, N, C], F16, isOutput=False)
    # weights pre-replicated on rows (64 -> 128) host-side for row packing
    Wq_d = nc.declare_dram_parameter("Wq", [128, CH], F16, isOutput=False)
    Wk_d = nc.declare_dram_parameter("Wk", [128, CH], F16, isOutput=False)
    Wv_d = nc.declare_dram_parameter("Wv", [128, CH], F16, isOutput=False)
    # Wout rearranged host-side: [p, cb, c] = Wout[cb*128+p, c]
    Wout_d = nc.declare_dram_parameter("Wout", [128, CB, C], F16, isOutput=False)
    ident_d = nc.declare_dram_parameter("ident", [128, 128], F16, isOutput=False)
    ones_d = nc.declare_dram_parameter("ones", [128, 128], F32, isOutput=False)
    out_d = nc.declare_dram_parameter("out", [2, C, N], F16, isOutput=True)

    with tile.TileContext(nc) as tc:
        with (
            tc.tile_pool(name="const", bufs=1) as constp,
            tc.tile_pool(name="io", bufs=2) as iop,
            tc.tile_pool(name="lt", bufs=2) as ltp,
            tc.tile_pool(name="embt", bufs=1) as embtp,
            tc.tile_pool(name="big", bufs=2) as bigp,
            tc.tile_pool(name="sT", bufs=1) as sTp,
            tc.tile_pool(name="kf", bufs=6) as kfp,
            tc.tile_pool(name="ctxs", bufs=1) as ctxp,
            tc.tile_pool(name="ot", bufs=2) as otp,
            tc.tile_pool(name="small", bufs=1) as smallp,
            tc.tile_pool(name="ps", bufs=1, space="PSUM") as psp,
        ):
            # ---- constants (all fp16 direct, no conversion) ----
            ident = constp.tile([128, 128], F16, tag="ident")
            nc.sync.dma_start(ident[:], ident_d[:])
            Wq_s = constp.tile([128, CH], F16, tag="Wq")
            nc.sync.dma_start(Wq_s[:], Wq_d[:])
            Wk_s = constp.tile([128, CH], F16, tag="Wk")
            nc.sync.dma_start(Wk_s[:], Wk_d[:])
            Wv_s = constp.tile([128, CH], F16, tag="Wv")
            nc.sync.dma_start(Wv_s[:], Wv_d[:])
            Wout_s = constp.tile([128, CB, C], F16, tag="Wout")
            nc.sync.dma_start(Wout_s[:], Wout_d[:])
            ones_f = iop.tile([128, 128], F32, tag="ones_f")
            nc.sync.dma_start(ones_f[:], ones_d[:])
            ones_r = constp.tile([128, 128], dt.float32r, tag="ones_r")
            nc.vector.tensor_copy(out=ones_r[:], in_=ones_f[:])
            ones16 = constp.tile([128, 1], F16, tag="ones16")
            nc.vector.tensor_copy(out=ones16[:], in_=ones_f[:, 0:1])

            # ---- PE warm-up: dense matmuls so HAM unthrottles early ----
            wu_ps = psp.tile([128, 512], F32, tag="pp", bufs=3)
            for w in range(24):
                nc.tensor.matmul(
                    wu_ps[:, 0:128], ident[:], ident[:],
                    start=True, stop=True)

            # ---- embT layout ----
            # embT_kv[c + 64*(db%2), db//2, n] ; embT_q[c + 64*inst, n]
            embT_kv = embtp.tile([128, B // 2, N], F16, tag="embT_kv")
            embT_q = embtp.tile([128, N], F16, tag="embT_q")

            # prefetch all emb loads up front (pair-interleaved on channels:
            # lt[p, nb, b*64+c] = src[b, nb*128+p, c])
            def load_pair(src2, nm):
                lt = ltp.tile([128, NB, 2, C], F16, tag="lt", name=nm)
                for b in range(2):
                    nc.sync.dma_start(
                        lt[:, :, b, :],
                        src2[b].rearrange("(nb p) c -> p nb c", p=128))
                return lt

            lt_q = load_pair(embq[:], "ltq")
            lt_kv = [load_pair(embkv[2 * p:2 * p + 2], f"ltkv{p}")
                     for p in range(B // 2)]

            def build_pair(lt, dst):  # dst: AP [128, N]
                for G in range(NG):
                    pt = psp.tile([128, 512], F16, tag="pp", bufs=3)
                    for j in range(4):
                        nc.tensor.transpose(
                            pt[:, j * 128:(j + 1) * 128],
                            lt[:, G * 4 + j, :, :], ident[:])
                    nc.vector.tensor_copy(
                        out=dst[:, G * 512:(G + 1) * 512], in_=pt[:])

            # ---- embT_q + phase Q (warms the PE early with real matmuls) ----
            build_pair(lt_q, embT_q[:])
            q_sb = [bigp.tile([128, NB, CH], F16, tag="big", name=f"q{i}")
                    for i in range(2)]
            for nb in range(NB):
                for i in range(2):
                    r = 64 * i
                    q_ps = psp.tile([128, 512], F32, tag="pp", bufs=3)
                    nc.tensor.matmul(
                        q_ps[:],
                        embT_q[r:r + 64, nb * 128:(nb + 1) * 128],
                        Wq_s[r:r + 64, :],
                        start=True, stop=True)
                    nc.vector.tensor_copy(out=q_sb[i][:, nb, :], in_=q_ps[:])

            sT = [sTp.tile([128, DB32, CH], F16, tag=f"sT{i}", name=f"sT{i}")
                  for i in range(2)]
            ssum = [smallp.tile([128, DB32], F32, tag=f"ssum{i}",
                                name=f"ssum{i}") for i in range(2)]
            ssq = [smallp.tile([128, DB32], F32, tag=f"ssq{i}",
                               name=f"ssq{i}") for i in range(2)]
            sqscr = smallp.tile([128, 512], F16, tag="sqscr")

            # ---- phase S: sT = kflat.T @ q, kf pipelined 2 steps ahead ----
            def proj_kf(step):
                db, nb = divmod(step, NB)
                r = 64 * (db % 2)
                kf_ps = psp.tile([128, 512], F32, tag="pp", bufs=3)
                nc.tensor.matmul(
                    kf_ps[:],
                    embT_kv[r:r + 64, db // 2, nb * 128:(nb + 1) * 128],
                    Wk_s[r:r + 64, :],
                    start=True, stop=True)
                kf = kfp.tile([128, 512], F16, tag="kf")
                nc.vector.tensor_copy(out=kf[:], in_=kf_ps[:])
                return kf

            def s_phase(inst, dbs, pipe):
                # pipe: dict carrying the kf lookahead across calls
                for db in dbs:
                    s_ps = [psp.tile([128, 512], F32, tag="sacc", bufs=5,
                                     name=f"sacc{inst}_{db}_{k}")
                            for k in range(4)]
                    bound = (dbs[-1] + 1) * NB - 1
                    for nb in range(NB):
                        step = db * NB + nb
                        # project kf in bursts of 4 to amortize the K=64/128
                        # LDWEIGHTS row-group switch; stay 2-6 steps ahead
                        if pipe["next"] <= min(step + 3, bound):
                            hi = min(pipe["next"] + 3, bound)
                            while pipe["next"] <= hi:
                                pipe[pipe["next"]] = proj_kf(pipe["next"])
                                pipe["next"] += 1
                        kf = pipe.pop(step)
                        for k in range(4):
                            nc.tensor.matmul(
                                s_ps[k][:],
                                kf[:, k * 128:(k + 1) * 128],
                                q_sb[inst][:, nb, :],
                                start=(nb == 0), stop=(nb == NB - 1))
                    for k in range(4):
                        dk = db * 4 + k
                        nc.scalar.activation(
                            sT[inst][:, dk, :], s_ps[k][:], AF.Copy,
                            accum_out=ssum[inst][:, dk:dk + 1])
                    # sumsq from the fp16 copies (banks already released)
                    for k in range(4):
                        dk = db * 4 + k
                        nc.scalar.activation(
                            sqscr[:], sT[inst][:, dk, :], AF.Square,
                            accum_out=ssq[inst][:, dk:dk + 1])

            # ---- stats / exp / den per instance ----
            stats = [smallp.tile([128, 8], F32, tag=f"stats{i}",
                                 name=f"stats{i}") for i in range(2)]
            inv_den = [smallp.tile([128, CB], F32, tag=f"invden{i}",
                                   name=f"invden{i}") for i in range(2)]

            def n_phase(inst):  # InstanceNorm stats
                red = smallp.tile([128, 2], F32, tag=f"red{inst}",
                                  name=f"red{inst}")
                nc.vector.tensor_reduce(
                    out=red[:, 0:1], in_=ssum[inst][:],
                    axis=mybir.AxisListType.X, op=ALU.add)
                nc.vector.tensor_reduce(
                    out=red[:, 1:2], in_=ssq[inst][:],
                    axis=mybir.AxisListType.X, op=ALU.add)
                red_r = smallp.tile([128, 2], dt.float32r, tag=f"redr{inst}",
                                    name=f"redr{inst}")
                nc.vector.tensor_copy(out=red_r[:], in_=red[:])
                ptr = psp.tile([128, 512], F32, tag="pp", bufs=3)
                nc.tensor.matmul(
                    ptr[:, 0:2], ones_r[:], red_r[:], start=True, stop=True)
                st = stats[inst]
                nc.scalar.activation(
                    st[:, 0:2], ptr[:, 0:2], AF.Copy, bias=0.0,
                    scale=1.0 / PLANE)
                mu, ex2 = st[:, 0:1], st[:, 1:2]
                musq, var = st[:, 2:3], st[:, 3:4]
                std, rstd, nmr = st[:, 4:5], st[:, 5:6], st[:, 6:7]
                nc.vector.tensor_tensor(out=musq, in0=mu, in1=mu, op=ALU.mult)
                nc.vector.tensor_tensor(out=var, in0=ex2, in1=musq,
                                        op=ALU.subtract)
                nc.vector.tensor_scalar_add(var, var, EPS)
                nc.scalar.activation(std, var, AF.Sqrt, bias=0.0)
                nc.vector.reciprocal(rstd, std)
                nc.vector.tensor_tensor(out=nmr, in0=mu, in1=rstd,
                                        op=ALU.mult)
                nc.scalar.mul(nmr, nmr, -1.0)

            def x_phase(inst):  # exp in place, chunked
                st = stats[inst]
                for G in range(NG):
                    nc.scalar.activation(
                        sT[inst][:, G * 4:(G + 1) * 4, :],
                        sT[inst][:, G * 4:(G + 1) * 4, :],
                        AF.Exp, bias=st[:, 6:7], scale=st[:, 5:6])

            def d_phase(inst):  # softmax denominator, dense 512-wide MMs
                den_ps = psp.tile([128, 512], F32, tag="pp", bufs=3)
                for dk in range(DB32):
                    nc.tensor.matmul(
                        den_ps[0:1, :], ones16[:], sT[inst][:, dk, :],
                        start=(dk == 0), stop=(dk == DB32 - 1))
                dr = smallp.tile([1, 512], F16, tag=f"denrow{inst}",
                                 name=f"denrow{inst}")
                nc.vector.tensor_copy(out=dr[:], in_=den_ps[0:1, :])
                # spread den[c] across partitions: K=1 matmuls per c-block
                spread = psp.tile([128, 512], F32, tag="pp", bufs=3)
                for cb in range(CB):
                    nc.tensor.matmul(
                        spread[:, cb:cb + 1],
                        dr[0:1, cb * 128:(cb + 1) * 128],
                        ones16[0:1, 0:1],
                        start=(cb == 0), stop=(cb == CB - 1))
                nc.vector.reciprocal(inv_den[inst][:], spread[:, 0:CB])

            # ---- emission: interleave embT_kv builds with s-phase(0) ----
            pipe = {"next": 0}
            for p in range(B // 2):
                build_pair(lt_kv[p], embT_kv[:, p, :])
                s_phase(0, [2 * p, 2 * p + 1], pipe)
            n_phase(0)
            x_phase(0)
            pipe = {"next": 0}
            s_phase(1, list(range(B)), pipe)
            d_phase(0)
            n_phase(1)
            x_phase(1)

            # ---- phase C: v on the fly + ctx + out; d_phase(1) spliced ----
            def ctx_pass(G, inst, va):
                ctx_ps = [psp.tile([128, 512], F32, tag="sacc", bufs=5,
                                   name=f"ctx{G}_{inst}_{cb}")
                          for cb in range(CB)]
                for db in range(B):
                    for chb in range(CB):
                        dk = db * 4 + chb
                        for cb in range(CB):
                            nc.tensor.matmul(
                                ctx_ps[cb][:],
                                sT[inst][:, dk, cb * 128:(cb + 1) * 128],
                                va[:, db, chb, :],
                                start=(dk == 0), stop=(dk == DB32 - 1))
                ctxs = ctxp.tile([128, CB, 512], F16, tag="ctxs")
                for cb in range(CB):
                    nc.scalar.activation(
                        ctxs[:, cb, :], ctx_ps[cb][:], AF.Copy,
                        scale=inv_den[inst][:, cb:cb + 1])
                out_ps = psp.tile([128, 512], F32, tag="pp", bufs=3)
                for cb in range(CB):
                    nc.tensor.matmul(
                        out_ps[0:C, :],
                        Wout_s[:, cb, :],
                        ctxs[:, cb, :],
                        start=(cb == 0), stop=(cb == CB - 1))
                ot = otp.tile([C, 512], F16, tag="ot")
                nc.vector.tensor_copy(out=ot[:], in_=out_ps[0:C, :])
                nc.sync.dma_start(
                    out_d[inst, :, G * 512:(G + 1) * 512], ot[:])

            for G in range(NG):
                va = bigp.tile([128, B, CB, 512], F16, tag="big",
                               name=f"va{G}")
                for db in range(B):
                    r = 64 * (db % 2)
                    for chb in range(CB):
                        v_ps = psp.tile([128, 512], F32, tag="pp", bufs=3)
                        nc.tensor.matmul(
                            v_ps[:],
                            Wv_s[r:r + 64, chb * 128:(chb + 1) * 128],
                            embT_kv[r:r + 64, db // 2,
                                    G * 512:(G + 1) * 512],
                            start=True, stop=True)
                        nc.vector.tensor_copy(out=va[:, db, chb, :],
                                              in_=v_ps[:])
                ctx_pass(G, 0, va)
                if G == 0:
                    d_phase(1)  # PE work while ACT finishes exp(1)
                ctx_pass(G, 1, va)

    nc.compile()
    return nc


def _get_nc():
    global _nc
    if _nc is None:
        _nc = _build()
    return _nc


def make_in_maps(emb, Wq, Wk, Wv, Wout):
    """Per-core input dicts (8 cores). Host-side fp16 casts + replication."""
    emb16 = np.ascontiguousarray(emb, dtype=np.float16)
    Wq16 = np.concatenate([Wq, Wq], axis=0).astype(np.float16)
    Wk16 = np.concatenate([Wk, Wk], axis=0).astype(np.float16)
    Wv16 = np.concatenate([Wv, Wv], axis=0).astype(np.float16)
    Wout16 = np.ascontiguousarray(
        Wout.reshape(CB, 128, C).transpose(1, 0, 2)).astype(np.float16)
    ident = np.eye(128, dtype=np.float16)
    ones = np.ones((128, 128), np.float32)
    emb_l, emb_u = emb16[:B], emb16[B:]
    in_maps = []
    for core in range(8):
        if core < 4:
            qb, kvb = emb_l[2 * core:2 * core + 2], emb_u
        else:
            j = core - 4
            qb, kvb = emb_u[2 * j:2 * j + 2], emb_l
        in_maps.append({
            "embq": np.ascontiguousarray(qb),
            "embkv": np.ascontiguousarray(kvb),
            "Wq": Wq16, "Wk": Wk16, "Wv": Wv16, "Wout": Wout16,
            "ident": ident, "ones": ones,
        })
    return in_maps


def kernel(emb, Wq, Wk, Wv, Wout):
    in_maps = make_in_maps(np.asarray(emb), np.asarray(Wq), np.asarray(Wk),
                           np.asarray(Wv), np.asarray(Wout))
    res = run_bass_kernel_spmd(_get_nc(), in_maps, list(range(8))).results
    out = np.empty((2 * B, N, C), np.float32)
    for core in range(8):
        o = res[core]["out"].transpose(0, 2, 1)  # [2, C, N] -> [2, N, C]
        if core < 4:
            out[2 * core:2 * core + 2] = o
        else:
            j = core - 4
            out[B + 2 * j:B + 2 * j + 2] = o
    return out


# revision 22
# speedup vs baseline: 1.6433x; 1.0172x over previous
"""Cross-attention (global, batch-flattened K/V) Trainium2 kernel, v3.

Problem: emb [16, 4096, 64]; two cross-attention halves:
  out_l2u = cross(q=emb[:8],  kv=emb[8:])   -> rows 0..7
  out_u2l = cross(q=emb[8:],  kv=emb[:8])   -> rows 8..15
cross(): q/k/v proj (64->512), s = einsum('bnc,nd->bcd', q, kflat),
InstanceNorm over (CH, B*CH) plane per b, softmax over d, ctx = a @ vflat^T,
out = ctx @ Wout.

Sharding: 16 (cross, q-batch) instances, 2 per core. Cores 0-3: q from
lower half (kv = upper), cores 4-7: q from upper (kv = lower). Both
instances on a core share the same kv half. No collectives.

Design: fp16 matmul operands (fp32 PSUM accum), sT layout (s stored
[d, c]: no aT transposes; softmax denominator via ones-matmuls), embT
of the kv half resident (built with paired [128,128] transposes, two
batches' channels stacked on partitions), v projected on the fly per
n-chunk (no DRAM scratch), kf projection software-pipelined two steps
ahead of its consumers, emission interleaved (embT builds between
s-phase batches; den between ctx passes) to keep the PE dense and warm.

Per-core phases:
  Eq/Q: embT_q + q projections        S(i): sT = kflat.T @ q, per batch,
  Ekv p: embT_kv pair p                     kf projected on the fly
  N(i): InstanceNorm stats             X(i): exp in place (ACT)
  D(i): softmax denom (ones matmuls)   C: per n-chunk: v proj + ctx + out
"""

import numpy as np
import concourse.bass as bass
import concourse.mybir as mybir
import concourse.tile as tile
from concourse import bacc
from concourse.bass_utils import run_bass_kernel_spmd

dt = mybir.dt
AF = mybir.ActivationFunctionType
ALU = mybir.AluOpType
F16 = dt.float16
F32 = dt.float32

B = 8            # batches per half
N = 4096         # sequence length
C = 64           # embedding channels
CH = 512         # num_heads * C
NB = N // 128    # 32 n-blocks
NG = N // 512    # 8 n-groups
CB = CH // 128   # 4 c-blocks
D = B * CH       # 4096 flattened kv dim
DB32 = D // 128  # 32 d-blocks
EPS = 1e-5
PLANE = float(CH * D)

_nc = None


def _build():
    nc = bacc.Bacc("TRN2", target_bir_lowering=False, debug=False, num_devices=8)

    embq = nc.declare_dram_parameter("embq", [2, N, C], F16, isOutput=False)
    embkv = nc.declare_dram_parameter("embkv", [B, N, C], F16, isOutput=False)
    # weights pre-replicated on rows (64 -> 128) host-side for row packing
    Wq_d = nc.declare_dram_parameter("Wq", [128, CH], F16, isOutput=False)
    Wk_d = nc.declare_dram_parameter("Wk", [128, CH], F16, isOutput=False)
    Wv_d = nc.declare_dram_parameter("Wv", [128, CH], F16, isOutput=False)
    # Wout rearranged host-side: [p, cb, c] = Wout[cb*128+p, c]
    Wout_d = nc.declare_dram_parameter("Wout", [128, CB, C], F16, isOutput=False)
    ident_d = nc.declare_dram_parameter("ident", [128, 128], F16, isOutput=False)
    ones_d = nc.declare_dram_parameter("ones", [128, 128], F32, isOutput=False)
    out_d = nc.declare_dram_parameter("out", [2, C, N], F16, isOutput=True)

    with tile.TileContext(nc) as tc:
        with (
            tc.tile_pool(name="const", bufs=1) as constp,
            tc.tile_pool(name="io", bufs=2) as iop,
            tc.tile_pool(name="lt", bufs=2) as ltp,
            tc.tile_pool(name="embt", bufs=1) as embtp,
            tc.tile_pool(name="big", bufs=2) as bigp,
            tc.tile_pool(name="sT", bufs=1) as sTp,
            tc.tile_pool(name="kf", bufs=6) as kfp,
            tc.tile_pool(name="ctxs", bufs=1) as ctxp,
            tc.tile_pool(name="ot", bufs=2) as otp,
            tc.tile_pool(name="small", bufs=1) as smallp,
            tc.tile_pool(name="ps", bufs=1, space="PSUM") as psp,
        ):
            # ---- constants (all fp16 direct, no conversion) ----
            ident = constp.tile([128, 128], F16, tag="ident")
            nc.sync.dma_start(ident[:], ident_d[:])

            # ---- PE warm-up: dense matmuls so HAM unthrottles early ----
            wu_ps = psp.tile([128, 512], F32, tag="pp", bufs=3)
            for w in range(24):
                nc.tensor.matmul(
                    wu_ps[:, 0:128], ident[:], ident[:],
                    start=True, stop=True)

            # ---- embT layout ----
            # embT_kv[c + 64*(db%2), db//2, n] ; embT_q[c + 64*inst, n]
            embT_kv = embtp.tile([128, B // 2, N], F16, tag="embT_kv")
            embT_q = embtp.tile([128, N], F16, tag="embT_q")

            # prefetch all emb loads up front (pair-interleaved on channels:
            # lt[p, nb, b*64+c] = src[b, nb*128+p, c])
            def load_pair(src2, nm):
                lt = ltp.tile([128, NB, 2, C], F16, tag="lt", name=nm)
                for b in range(2):
                    nc.sync.dma_start(
                        lt[:, :, b, :],
                        src2[b].rearrange("(nb p) c -> p nb c", p=128))
                return lt

            lt_q = load_pair(embq[:], "ltq")
            lt_kv = [load_pair(embkv[2 * p:2 * p + 2], f"ltkv{p}")
                     for p in range(B // 2)]

            Wq_s = constp.tile([128, CH], F16, tag="Wq")
            nc.sync.dma_start(Wq_s[:], Wq_d[:])
            Wk_s = constp.tile([128, CH], F16, tag="Wk")
            nc.sync.dma_start(Wk_s[:], Wk_d[:])
            Wv_s = constp.tile([128, CH], F16, tag="Wv")
            nc.sync.dma_start(Wv_s[:], Wv_d[:])
            Wout_s = constp.tile([128, CB, C], F16, tag="Wout")
            nc.sync.dma_start(Wout_s[:], Wout_d[:])
            ones_f = iop.tile([128, 128], F32, tag="ones_f")
            nc.sync.dma_start(ones_f[:], ones_d[:])
            ones_r = constp.tile([128, 128], dt.float32r, tag="ones_r")
            nc.vector.tensor_copy(out=ones_r[:], in_=ones_f[:])
            ones16 = constp.tile([128, 1], F16, tag="ones16")
            nc.vector.tensor_copy(out=ones16[:], in_=ones_f[:, 0:1])

            def emit_group(lt, dst, G):  # one 512-n group of transposes
                pt = psp.tile([128, 512], F16, tag="pp", bufs=3)
                for j in range(4):
                    nc.tensor.transpose(
                        pt[:, j * 128:(j + 1) * 128],
                        lt[:, G * 4 + j, :, :], ident[:])
                nc.vector.tensor_copy(
                    out=dst[:, G * 512:(G + 1) * 512], in_=pt[:])

            # ---- embT_q build fused with q projections (PE stays dense) ----
            q_sb = [bigp.tile([128, NB, CH], F16, tag="big", name=f"q{i}")
                    for i in range(2)]
            emit_group(lt_q, embT_q[:], 0)
            emit_group(lt_q, embT_q[:], 1)
            for G in range(NG):
                for nb in range(G * 4, G * 4 + 4):
                    for i in range(2):
                        r = 64 * i
                        q_ps = psp.tile([128, 512], F32, tag="pp", bufs=3)
                        nc.tensor.matmul(
                            q_ps[:],
                            embT_q[r:r + 64, nb * 128:(nb + 1) * 128],
                            Wq_s[r:r + 64, :],
                            start=True, stop=True)
                        nc.vector.tensor_copy(out=q_sb[i][:, nb, :],
                                              in_=q_ps[:])
                if G + 2 < NG:
                    emit_group(lt_q, embT_q[:], G + 2)

            sT = [sTp.tile([128, DB32, CH], F16, tag=f"sT{i}", name=f"sT{i}")
                  for i in range(2)]
            ssum = [smallp.tile([128, DB32], F32, tag=f"ssum{i}",
                                name=f"ssum{i}") for i in range(2)]
            ssq = [smallp.tile([128, DB32], F32, tag=f"ssq{i}",
                               name=f"ssq{i}") for i in range(2)]
            sqscr = smallp.tile([128, 512], F16, tag="sqscr")

            # ---- phase S: sT = kflat.T @ q, kf pipelined 2 steps ahead ----
            def proj_kf(step):
                db, nb = divmod(step, NB)
                r = 64 * (db % 2)
                kf_ps = psp.tile([128, 512], F32, tag="pp", bufs=3)
                nc.tensor.matmul(
                    kf_ps[:],
                    embT_kv[r:r + 64, db // 2, nb * 128:(nb + 1) * 128],
                    Wk_s[r:r + 64, :],
                    start=True, stop=True)
                kf = kfp.tile([128, 512], F16, tag="kf")
                nc.vector.tensor_copy(out=kf[:], in_=kf_ps[:])
                return kf

            def s_phase(inst, dbs, pipe, build=None):
                # pipe: dict carrying the kf lookahead across calls
                for db in dbs:
                    s_ps = [psp.tile([128, 512], F32, tag="sacc", bufs=5,
                                     name=f"sacc{inst}_{db}_{k}")
                            for k in range(4)]
                    bound = (dbs[-1] + 1) * NB - 1
                    for nb in range(NB):
                        if build is not None and db == dbs[0] and nb % 4 == 0:
                            lt, dst = build
                            if nb == 0:
                                emit_group(lt, dst, 0)
                                emit_group(lt, dst, 1)
                            g = nb // 4 + 2
                            if g < NG:
                                emit_group(lt, dst, g)
                        step = db * NB + nb
                        # project kf in bursts of 4 to amortize the K=64/128
                        # LDWEIGHTS row-group switch; stay 2-6 steps ahead
                        if pipe["next"] <= min(step + 3, bound):
                            hi = min(pipe["next"] + 3, bound)
                            while pipe["next"] <= hi:
                                pipe[pipe["next"]] = proj_kf(pipe["next"])
                                pipe["next"] += 1
                        kf = pipe.pop(step)
                        for k in range(4):
                            nc.tensor.matmul(
                                s_ps[k][:],
                                kf[:, k * 128:(k + 1) * 128],
                                q_sb[inst][:, nb, :],
                                start=(nb == 0), stop=(nb == NB - 1))
                    for k in range(4):
                        dk = db * 4 + k
                        nc.scalar.activation(
                            sT[inst][:, dk, :], s_ps[k][:], AF.Copy,
                            accum_out=ssum[inst][:, dk:dk + 1])
                    # sumsq from the fp16 copies (banks already released)
                    for k in range(4):
                        dk = db * 4 + k
                        nc.scalar.activation(
                            sqscr[:], sT[inst][:, dk, :], AF.Square,
                            accum_out=ssq[inst][:, dk:dk + 1])

            # ---- stats / exp / den per instance ----
            stats = [smallp.tile([128, 8], F32, tag=f"stats{i}",
                                 name=f"stats{i}") for i in range(2)]
            inv_den = [smallp.tile([128, CB], F32, tag=f"invden{i}",
                                   name=f"invden{i}") for i in range(2)]

            def n_phase(inst):  # InstanceNorm stats
                red = smallp.tile([128, 2], F32, tag=f"red{inst}",
                                  name=f"red{inst}")
                nc.vector.tensor_reduce(
                    out=red[:, 0:1], in_=ssum[inst][:],
                    axis=mybir.AxisListType.X, op=ALU.add)
                nc.vector.tensor_reduce(
                    out=red[:, 1:2], in_=ssq[inst][:],
                    axis=mybir.AxisListType.X, op=ALU.add)
                red_r = smallp.tile([128, 2], dt.float32r, tag=f"redr{inst}",
                                    name=f"redr{inst}")
                nc.vector.tensor_copy(out=red_r[:], in_=red[:])
                ptr = psp.tile([128, 512], F32, tag="pp", bufs=3)
                nc.tensor.matmul(
                    ptr[:, 0:2], ones_r[:], red_r[:], start=True, stop=True)
                st = stats[inst]
                nc.scalar.activation(
                    st[:, 0:2], ptr[:, 0:2], AF.Copy, bias=0.0,
                    scale=1.0 / PLANE)
                mu, ex2 = st[:, 0:1], st[:, 1:2]
                musq, var = st[:, 2:3], st[:, 3:4]
                std, rstd, nmr = st[:, 4:5], st[:, 5:6], st[:, 6:7]
                nc.vector.tensor_tensor(out=musq, in0=mu, in1=mu, op=ALU.mult)
                nc.vector.tensor_tensor(out=var, in0=ex2, in1=musq,
                                        op=ALU.subtract)
                nc.vector.tensor_scalar_add(var, var, EPS)
                nc.scalar.activation(std, var, AF.Sqrt, bias=0.0)
                nc.vector.reciprocal(rstd, std)
                nc.vector.tensor_tensor(out=nmr, in0=mu, in1=rstd,
                                        op=ALU.mult)
                nc.scalar.mul(nmr, nmr, -1.0)

            def x_phase(inst):  # exp in place, chunked
                st = stats[inst]
                for G in range(NG):
                    nc.scalar.activation(
                        sT[inst][:, G * 4:(G + 1) * 4, :],
                        sT[inst][:, G * 4:(G + 1) * 4, :],
                        AF.Exp, bias=st[:, 6:7], scale=st[:, 5:6])

            def d_phase(inst):  # softmax denominator, dense 512-wide MMs
                den_ps = psp.tile([128, 512], F32, tag="pp", bufs=3)
                for dk in range(DB32):
                    nc.tensor.matmul(
                        den_ps[0:1, :], ones16[:], sT[inst][:, dk, :],
                        start=(dk == 0), stop=(dk == DB32 - 1))
                dr = smallp.tile([1, 512], F16, tag=f"denrow{inst}",
                                 name=f"denrow{inst}")
                nc.vector.tensor_copy(out=dr[:], in_=den_ps[0:1, :])
                # spread den[c] across partitions: K=1 matmuls per c-block
                spread = psp.tile([128, 512], F32, tag="pp", bufs=3)
                for cb in range(CB):
                    nc.tensor.matmul(
                        spread[:, cb:cb + 1],
                        dr[0:1, cb * 128:(cb + 1) * 128],
                        ones16[0:1, 0:1],
                        start=(cb == 0), stop=(cb == CB - 1))
                nc.vector.reciprocal(inv_den[inst][:], spread[:, 0:CB])

            # ---- emission: interleave embT_kv builds with s-phase(0) ----
            pipe = {"next": 0}
            for p in range(B // 2):
                s_phase(0, [2 * p, 2 * p + 1], pipe,
                        build=(lt_kv[p], embT_kv[:, p, :]))
            n_phase(0)
            x_phase(0)
            pipe = {"next": 0}
            s_phase(1, list(range(B)), pipe)
            d_phase(0)
            n_phase(1)
            x_phase(1)

            # ---- phase C: v on the fly + ctx + out; d_phase(1) spliced ----
            def vp(G, va, db):  # project v for one batch / n-chunk
                r = 64 * (db % 2)
                for chb in range(CB):
                    v_ps = psp.tile([128, 512], F32, tag="pp", bufs=3)
                    nc.tensor.matmul(
                        v_ps[:],
                        Wv_s[r:r + 64, chb * 128:(chb + 1) * 128],
                        embT_kv[r:r + 64, db // 2, G * 512:(G + 1) * 512],
                        start=True, stop=True)
                    nc.vector.tensor_copy(out=va[:, db, chb, :],
                                          in_=v_ps[:])

            def ctx_pass(G, inst, va, fuse_vp):
                ctx_ps = [psp.tile([128, 512], F32, tag="sacc", bufs=5,
                                   name=f"ctx{G}_{inst}_{cb}")
                          for cb in range(CB)]
                for db in range(B):
                    if fuse_vp and db + 2 < B:
                        vp(G, va, db + 2)
                    for chb in range(CB):
                        dk = db * 4 + chb
                        for cb in range(CB):
                            nc.tensor.matmul(
                                ctx_ps[cb][:],
                                sT[inst][:, dk, cb * 128:(cb + 1) * 128],
                                va[:, db, chb, :],
                                start=(dk == 0), stop=(dk == DB32 - 1))
                ctxs = ctxp.tile([128, CB, 512], F16, tag="ctxs")
                for cb in range(CB):
                    nc.scalar.activation(
                        ctxs[:, cb, :], ctx_ps[cb][:], AF.Copy,
                        scale=inv_den[inst][:, cb:cb + 1])
                out_ps = psp.tile([128, 512], F32, tag="pp", bufs=3)
                for cb in range(CB):
                    nc.tensor.matmul(
                        out_ps[0:C, :],
                        Wout_s[:, cb, :],
                        ctxs[:, cb, :],
                        start=(cb == 0), stop=(cb == CB - 1))
                ot = otp.tile([C, 512], F16, tag="ot")
                nc.vector.tensor_copy(out=ot[:], in_=out_ps[0:C, :])
                nc.sync.dma_start(
                    out_d[inst, :, G * 512:(G + 1) * 512], ot[:])

            for G in range(NG):
                va = bigp.tile([128, B, CB, 512], F16, tag="big",
                               name=f"va{G}")
                vp(G, va, 0)
                vp(G, va, 1)
                ctx_pass(G, 0, va, fuse_vp=True)
                if G == 0:
                    d_phase(1)  # PE work while ACT finishes exp(1)
                ctx_pass(G, 1, va, fuse_vp=False)

    nc.compile()
    return nc


def _get_nc():
    global _nc
    if _nc is None:
        _nc = _build()
    return _nc


def make_in_maps(emb, Wq, Wk, Wv, Wout):
    """Per-core input dicts (8 cores). Host-side fp16 casts + replication."""
    emb16 = np.ascontiguousarray(emb, dtype=np.float16)
    Wq16 = np.concatenate([Wq, Wq], axis=0).astype(np.float16)
    Wk16 = np.concatenate([Wk, Wk], axis=0).astype(np.float16)
    Wv16 = np.concatenate([Wv, Wv], axis=0).astype(np.float16)
    Wout16 = np.ascontiguousarray(
        Wout.reshape(CB, 128, C).transpose(1, 0, 2)).astype(np.float16)
    ident = np.eye(128, dtype=np.float16)
    ones = np.ones((128, 128), np.float32)
    emb_l, emb_u = emb16[:B], emb16[B:]
    in_maps = []
    for core in range(8):
        if core < 4:
            qb, kvb = emb_l[2 * core:2 * core + 2], emb_u
        else:
            j = core - 4
            qb, kvb = emb_u[2 * j:2 * j + 2], emb_l
        in_maps.append({
            "embq": np.ascontiguousarray(qb),
            "embkv": np.ascontiguousarray(kvb),
            "Wq": Wq16, "Wk": Wk16, "Wv": Wv16, "Wout": Wout16,
            "ident": ident, "ones": ones,
        })
    return in_maps


def kernel(emb, Wq, Wk, Wv, Wout):
    in_maps = make_in_maps(np.asarray(emb), np.asarray(Wq), np.asarray(Wk),
                           np.asarray(Wv), np.asarray(Wout))
    res = run_bass_kernel_spmd(_get_nc(), in_maps, list(range(8))).results
    out = np.empty((2 * B, N, C), np.float32)
    for core in range(8):
        o = res[core]["out"].transpose(0, 2, 1)  # [2, C, N] -> [2, N, C]
        if core < 4:
            out[2 * core:2 * core + 2] = o
        else:
            j = core - 4
            out[B + 2 * j:B + 2 * j + 2] = o
    return out


# revision 23
# speedup vs baseline: 1.6662x; 1.0139x over previous
"""Cross-attention (global, batch-flattened K/V) Trainium2 kernel, v3.

Problem: emb [16, 4096, 64]; two cross-attention halves:
  out_l2u = cross(q=emb[:8],  kv=emb[8:])   -> rows 0..7
  out_u2l = cross(q=emb[8:],  kv=emb[:8])   -> rows 8..15
cross(): q/k/v proj (64->512), s = einsum('bnc,nd->bcd', q, kflat),
InstanceNorm over (CH, B*CH) plane per b, softmax over d, ctx = a @ vflat^T,
out = ctx @ Wout.

Sharding: 16 (cross, q-batch) instances, 2 per core. Cores 0-3: q from
lower half (kv = upper), cores 4-7: q from upper (kv = lower). Both
instances on a core share the same kv half. No collectives.

Design: fp16 matmul operands (fp32 PSUM accum), sT layout (s stored
[d, c]: no aT transposes; softmax denominator via ones-matmuls), embT
of the kv half resident (built with paired [128,128] transposes, two
batches' channels stacked on partitions), v projected on the fly per
n-chunk (no DRAM scratch), kf projection software-pipelined two steps
ahead of its consumers, emission interleaved (embT builds between
s-phase batches; den between ctx passes) to keep the PE dense and warm.

Per-core phases:
  Eq/Q: embT_q + q projections        S(i): sT = kflat.T @ q, per batch,
  Ekv p: embT_kv pair p                     kf projected on the fly
  N(i): InstanceNorm stats             X(i): exp in place (ACT)
  D(i): softmax denom (ones matmuls)   C: per n-chunk: v proj + ctx + out
"""

import numpy as np
import concourse.bass as bass
import concourse.mybir as mybir
import concourse.tile as tile
from concourse import bacc
from concourse.bass_utils import run_bass_kernel_spmd

dt = mybir.dt
AF = mybir.ActivationFunctionType
ALU = mybir.AluOpType
F16 = dt.float16
F32 = dt.float32

B = 8            # batches per half
N = 4096         # sequence length
C = 64           # embedding channels
CH = 512         # num_heads * C
NB = N // 128    # 32 n-blocks
NG = N // 512    # 8 n-groups
CB = CH // 128   # 4 c-blocks
D = B * CH       # 4096 flattened kv dim
DB32 = D // 128  # 32 d-blocks
EPS = 1e-5
PLANE = float(CH * D)

_nc = None


def _build():
    nc = bacc.Bacc("TRN2", target_bir_lowering=False, debug=False, num_devices=8)

    embq = nc.declare_dram_parameter("embq", [2, N, C], F16, isOutput=False)
    embkv = nc.declare_dram_parameter("embkv", [B, N, C], F16, isOutput=False)
    # weights pre-replicated on rows (64 -> 128) host-side for row packing
    Wq_d = nc.declare_dram_parameter("Wq", [128, CH], F16, isOutput=False)
    Wk_d = nc.declare_dram_parameter("Wk", [128, CH], F16, isOutput=False)
    Wv_d = nc.declare_dram_parameter("Wv", [128, CH], F16, isOutput=False)
    # Wout rearranged host-side: [p, cb, c] = Wout[cb*128+p, c]
    Wout_d = nc.declare_dram_parameter("Wout", [128, CB, C], F16, isOutput=False)
    ident_d = nc.declare_dram_parameter("ident", [128, 128], F16, isOutput=False)
    ones_d = nc.declare_dram_parameter("ones", [128, 128], F32, isOutput=False)
    out_d = nc.declare_dram_parameter("out", [2, C, N], F16, isOutput=True)

    with tile.TileContext(nc) as tc:
        with (
            tc.tile_pool(name="const", bufs=1) as constp,
            tc.tile_pool(name="io", bufs=2) as iop,
            tc.tile_pool(name="lt", bufs=2) as ltp,
            tc.tile_pool(name="embt", bufs=1) as embtp,
            tc.tile_pool(name="big", bufs=2) as bigp,
            tc.tile_pool(name="sT", bufs=1) as sTp,
            tc.tile_pool(name="kf", bufs=6) as kfp,
            tc.tile_pool(name="ctxs", bufs=1) as ctxp,
            tc.tile_pool(name="ot", bufs=2) as otp,
            tc.tile_pool(name="small", bufs=1) as smallp,
            tc.tile_pool(name="ps", bufs=1, space="PSUM") as psp,
        ):
            # ---- constants (all fp16 direct, no conversion) ----
            ident = constp.tile([128, 128], F16, tag="ident")
            nc.sync.dma_start(ident[:], ident_d[:])

            # ---- PE warm-up: dense matmuls so HAM unthrottles early ----
            wu_ps = psp.tile([128, 512], F32, tag="pp", bufs=3)
            for w in range(56):
                nc.tensor.matmul(
                    wu_ps[:, 0:128], ident[:], ident[:],
                    start=True, stop=True)

            # ---- embT layout ----
            # embT_kv[c + 64*(db%2), db//2, n] ; embT_q[c + 64*inst, n]
            embT_kv = embtp.tile([128, B // 2, N], F16, tag="embT_kv")
            embT_q = embtp.tile([128, N], F16, tag="embT_q")

            # prefetch all emb loads up front (pair-interleaved on channels:
            # lt[p, nb, b*64+c] = src[b, nb*128+p, c])
            def load_pair(src2, nm):
                lt = ltp.tile([128, NB, 2, C], F16, tag="lt", name=nm)
                for b in range(2):
                    nc.sync.dma_start(
                        lt[:, :, b, :],
                        src2[b].rearrange("(nb p) c -> p nb c", p=128))
                return lt

            lt_q = load_pair(embq[:], "ltq")
            lt_kv = [load_pair(embkv[2 * p:2 * p + 2], f"ltkv{p}")
                     for p in range(B // 2)]

            Wq_s = constp.tile([128, CH], F16, tag="Wq")
            nc.sync.dma_start(Wq_s[:], Wq_d[:])
            Wk_s = constp.tile([128, CH], F16, tag="Wk")
            nc.sync.dma_start(Wk_s[:], Wk_d[:])
            Wv_s = constp.tile([128, CH], F16, tag="Wv")
            nc.sync.dma_start(Wv_s[:], Wv_d[:])
            Wout_s = constp.tile([128, CB, C], F16, tag="Wout")
            nc.sync.dma_start(Wout_s[:], Wout_d[:])
            ones_f = iop.tile([128, 128], F32, tag="ones_f")
            nc.sync.dma_start(ones_f[:], ones_d[:])
            ones_r = constp.tile([128, 128], dt.float32r, tag="ones_r")
            nc.vector.tensor_copy(out=ones_r[:], in_=ones_f[:])
            ones16 = constp.tile([128, 1], F16, tag="ones16")
            nc.vector.tensor_copy(out=ones16[:], in_=ones_f[:, 0:1])

            def emit_group(lt, dst, G):  # one 512-n group of transposes
                pt = psp.tile([128, 512], F16, tag="pp", bufs=3)
                for j in range(4):
                    nc.tensor.transpose(
                        pt[:, j * 128:(j + 1) * 128],
                        lt[:, G * 4 + j, :, :], ident[:])
                nc.vector.tensor_copy(
                    out=dst[:, G * 512:(G + 1) * 512], in_=pt[:])

            # ---- embT_q build fused with q projections (PE stays dense) ----
            q_sb = [bigp.tile([128, NB, CH], F16, tag="big", name=f"q{i}")
                    for i in range(2)]
            emit_group(lt_q, embT_q[:], 0)
            emit_group(lt_q, embT_q[:], 1)
            for G in range(NG):
                for nb in range(G * 4, G * 4 + 4):
                    for i in range(2):
                        r = 64 * i
                        q_ps = psp.tile([128, 512], F32, tag="pp", bufs=3)
                        nc.tensor.matmul(
                            q_ps[:],
                            embT_q[r:r + 64, nb * 128:(nb + 1) * 128],
                            Wq_s[r:r + 64, :],
                            start=True, stop=True)
                        nc.vector.tensor_copy(out=q_sb[i][:, nb, :],
                                              in_=q_ps[:])
                if G + 2 < NG:
                    emit_group(lt_q, embT_q[:], G + 2)

            sT = [sTp.tile([128, DB32, CH], F16, tag=f"sT{i}", name=f"sT{i}")
                  for i in range(2)]
            ssum = [smallp.tile([128, DB32], F32, tag=f"ssum{i}",
                                name=f"ssum{i}") for i in range(2)]
            ssq = [smallp.tile([128, DB32], F32, tag=f"ssq{i}",
                               name=f"ssq{i}") for i in range(2)]
            sqscr = smallp.tile([128, 512], F16, tag="sqscr")

            # ---- phase S: sT = kflat.T @ q, kf pipelined 2 steps ahead ----
            def proj_kf(step):
                db, nb = divmod(step, NB)
                r = 64 * (db % 2)
                kf_ps = psp.tile([128, 512], F32, tag="pp", bufs=3)
                nc.tensor.matmul(
                    kf_ps[:],
                    embT_kv[r:r + 64, db // 2, nb * 128:(nb + 1) * 128],
                    Wk_s[r:r + 64, :],
                    start=True, stop=True)
                kf = kfp.tile([128, 512], F16, tag="kf")
                nc.vector.tensor_copy(out=kf[:], in_=kf_ps[:])
                return kf

            def s_phase(inst, dbs, pipe, build=None):
                # pipe: dict carrying the kf lookahead across calls
                for db in dbs:
                    s_ps = [psp.tile([128, 512], F32, tag="sacc", bufs=5,
                                     name=f"sacc{inst}_{db}_{k}")
                            for k in range(4)]
                    bound = (dbs[-1] + 1) * NB - 1
                    for nb in range(NB):
                        if build is not None and db == dbs[0] and nb % 4 == 0:
                            lt, dst = build
                            if nb == 0:
                                emit_group(lt, dst, 0)
                                emit_group(lt, dst, 1)
                            g = nb // 4 + 2
                            if g < NG:
                                emit_group(lt, dst, g)
                        step = db * NB + nb
                        # project kf in bursts of 4 to amortize the K=64/128
                        # LDWEIGHTS row-group switch; stay 2-6 steps ahead
                        if pipe["next"] <= min(step + 3, bound):
                            hi = min(pipe["next"] + 3, bound)
                            while pipe["next"] <= hi:
                                pipe[pipe["next"]] = proj_kf(pipe["next"])
                                pipe["next"] += 1
                        kf = pipe.pop(step)
                        for k in range(4):
                            nc.tensor.matmul(
                                s_ps[k][:],
                                kf[:, k * 128:(k + 1) * 128],
                                q_sb[inst][:, nb, :],
                                start=(nb == 0), stop=(nb == NB - 1))
                    for k in range(4):
                        dk = db * 4 + k
                        nc.scalar.activation(
                            sT[inst][:, dk, :], s_ps[k][:], AF.Copy,
                            accum_out=ssum[inst][:, dk:dk + 1])
                    # sumsq from the fp16 copies (banks already released)
                    for k in range(4):
                        dk = db * 4 + k
                        nc.scalar.activation(
                            sqscr[:], sT[inst][:, dk, :], AF.Square,
                            accum_out=ssq[inst][:, dk:dk + 1])

            # ---- stats / exp / den per instance ----
            stats = [smallp.tile([128, 8], F32, tag=f"stats{i}",
                                 name=f"stats{i}") for i in range(2)]
            inv_den = [smallp.tile([128, CB], F32, tag=f"invden{i}",
                                   name=f"invden{i}") for i in range(2)]

            def n_phase(inst):  # InstanceNorm stats
                red = smallp.tile([128, 2], F32, tag=f"red{inst}",
                                  name=f"red{inst}")
                nc.vector.tensor_reduce(
                    out=red[:, 0:1], in_=ssum[inst][:],
                    axis=mybir.AxisListType.X, op=ALU.add)
                nc.vector.tensor_reduce(
                    out=red[:, 1:2], in_=ssq[inst][:],
                    axis=mybir.AxisListType.X, op=ALU.add)
                red_r = smallp.tile([128, 2], dt.float32r, tag=f"redr{inst}",
                                    name=f"redr{inst}")
                nc.vector.tensor_copy(out=red_r[:], in_=red[:])
                ptr = psp.tile([128, 512], F32, tag="pp", bufs=3)
                nc.tensor.matmul(
                    ptr[:, 0:2], ones_r[:], red_r[:], start=True, stop=True)
                st = stats[inst]
                nc.scalar.activation(
                    st[:, 0:2], ptr[:, 0:2], AF.Copy, bias=0.0,
                    scale=1.0 / PLANE)
                mu, ex2 = st[:, 0:1], st[:, 1:2]
                musq, var = st[:, 2:3], st[:, 3:4]
                std, rstd, nmr = st[:, 4:5], st[:, 5:6], st[:, 6:7]
                nc.vector.tensor_tensor(out=musq, in0=mu, in1=mu, op=ALU.mult)
                nc.vector.tensor_tensor(out=var, in0=ex2, in1=musq,
                                        op=ALU.subtract)
                nc.vector.tensor_scalar_add(var, var, EPS)
                nc.scalar.activation(std, var, AF.Sqrt, bias=0.0)
                nc.vector.reciprocal(rstd, std)
                nc.vector.tensor_tensor(out=nmr, in0=mu, in1=rstd,
                                        op=ALU.mult)
                nc.scalar.mul(nmr, nmr, -1.0)

            def x_phase(inst):  # exp in place, chunked
                st = stats[inst]
                for G in range(NG):
                    nc.scalar.activation(
                        sT[inst][:, G * 4:(G + 1) * 4, :],
                        sT[inst][:, G * 4:(G + 1) * 4, :],
                        AF.Exp, bias=st[:, 6:7], scale=st[:, 5:6])

            def d_phase(inst):  # softmax denominator, dense 512-wide MMs
                den_ps = psp.tile([128, 512], F32, tag="pp", bufs=3)
                for dk in range(DB32):
                    nc.tensor.matmul(
                        den_ps[0:1, :], ones16[:], sT[inst][:, dk, :],
                        start=(dk == 0), stop=(dk == DB32 - 1))
                dr = smallp.tile([1, 512], F16, tag=f"denrow{inst}",
                                 name=f"denrow{inst}")
                nc.vector.tensor_copy(out=dr[:], in_=den_ps[0:1, :])
                # spread den[c] across partitions: K=1 matmuls per c-block
                spread = psp.tile([128, 512], F32, tag="pp", bufs=3)
                for cb in range(CB):
                    nc.tensor.matmul(
                        spread[:, cb:cb + 1],
                        dr[0:1, cb * 128:(cb + 1) * 128],
                        ones16[0:1, 0:1],
                        start=(cb == 0), stop=(cb == CB - 1))
                nc.vector.reciprocal(inv_den[inst][:], spread[:, 0:CB])

            # ---- emission: interleave embT_kv builds with s-phase(0) ----
            pipe = {"next": 0}
            for p in range(B // 2):
                s_phase(0, [2 * p, 2 * p + 1], pipe,
                        build=(lt_kv[p], embT_kv[:, p, :]))
            pipe = {"next": 0}
            s_phase(1, [0], pipe)     # dense PE work over s0's drain chain
            n_phase(0)
            x_phase(0)
            s_phase(1, list(range(1, B)), pipe)
            d_phase(0)                # dense PE work over s1's drain chain
            n_phase(1)
            x_phase(1)

            # ---- phase C: v on the fly + ctx + out; d_phase(1) spliced ----
            def vp(G, va, db):  # project v for one batch / n-chunk
                r = 64 * (db % 2)
                for chb in range(CB):
                    v_ps = psp.tile([128, 512], F32, tag="pp", bufs=3)
                    nc.tensor.matmul(
                        v_ps[:],
                        Wv_s[r:r + 64, chb * 128:(chb + 1) * 128],
                        embT_kv[r:r + 64, db // 2, G * 512:(G + 1) * 512],
                        start=True, stop=True)
                    nc.vector.tensor_copy(out=va[:, db, chb, :],
                                          in_=v_ps[:])

            def ctx_pass(G, inst, va, fuse_vp):
                ctx_ps = [psp.tile([128, 512], F32, tag="sacc", bufs=5,
                                   name=f"ctx{G}_{inst}_{cb}")
                          for cb in range(CB)]
                for db in range(B):
                    if fuse_vp and db + 2 < B:
                        vp(G, va, db + 2)
                    for chb in range(CB):
                        dk = db * 4 + chb
                        for cb in range(CB):
                            nc.tensor.matmul(
                                ctx_ps[cb][:],
                                sT[inst][:, dk, cb * 128:(cb + 1) * 128],
                                va[:, db, chb, :],
                                start=(dk == 0), stop=(dk == DB32 - 1))
                ctxs = ctxp.tile([128, CB, 512], F16, tag="ctxs")
                for cb in range(CB):
                    nc.scalar.activation(
                        ctxs[:, cb, :], ctx_ps[cb][:], AF.Copy,
                        scale=inv_den[inst][:, cb:cb + 1])
                out_ps = psp.tile([128, 512], F32, tag="pp", bufs=3)
                for cb in range(CB):
                    nc.tensor.matmul(
                        out_ps[0:C, :],
                        Wout_s[:, cb, :],
                        ctxs[:, cb, :],
                        start=(cb == 0), stop=(cb == CB - 1))
                ot = otp.tile([C, 512], F16, tag="ot")
                nc.vector.tensor_copy(out=ot[:], in_=out_ps[0:C, :])
                nc.sync.dma_start(
                    out_d[inst, :, G * 512:(G + 1) * 512], ot[:])

            for G in range(NG):
                va = bigp.tile([128, B, CB, 512], F16, tag="big",
                               name=f"va{G}")
                vp(G, va, 0)
                vp(G, va, 1)
                ctx_pass(G, 0, va, fuse_vp=True)
                if G == 0:
                    d_phase(1)  # PE work while ACT finishes exp(1)
                ctx_pass(G, 1, va, fuse_vp=False)

    nc.compile()
    return nc


def _get_nc():
    global _nc
    if _nc is None:
        _nc = _build()
    return _nc


def make_in_maps(emb, Wq, Wk, Wv, Wout):
    """Per-core input dicts (8 cores). Host-side fp16 casts + replication."""
    emb16 = np.ascontiguousarray(emb, dtype=np.float16)
    Wq16 = np.concatenate([Wq, Wq], axis=0).astype(np.float16)
    Wk16 = np.concatenate([Wk, Wk], axis=0).astype(np.float16)
    Wv16 = np.concatenate([Wv, Wv], axis=0).astype(np.float16)
    Wout16 = np.ascontiguousarray(
        Wout.reshape(CB, 128, C).transpose(1, 0, 2)).astype(np.float16)
    ident = np.eye(128, dtype=np.float16)
    ones = np.ones((128, 128), np.float32)
    emb_l, emb_u = emb16[:B], emb16[B:]
    in_maps = []
    for core in range(8):
        if core < 4:
            qb, kvb = emb_l[2 * core:2 * core + 2], emb_u
        else:
            j = core - 4
            qb, kvb = emb_u[2 * j:2 * j + 2], emb_l
        in_maps.append({
            "embq": np.ascontiguousarray(qb),
            "embkv": np.ascontiguousarray(kvb),
            "Wq": Wq16, "Wk": Wk16, "Wv": Wv16, "Wout": Wout16,
            "ident": ident, "ones": ones,
        })
    return in_maps


def kernel(emb, Wq, Wk, Wv, Wout):
    in_maps = make_in_maps(np.asarray(emb), np.asarray(Wq), np.asarray(Wk),
                           np.asarray(Wv), np.asarray(Wout))
    res = run_bass_kernel_spmd(_get_nc(), in_maps, list(range(8))).results
    out = np.empty((2 * B, N, C), np.float32)
    for core in range(8):
        o = res[core]["out"].transpose(0, 2, 1)  # [2, C, N] -> [2, N, C]
        if core < 4:
            out[2 * core:2 * core + 2] = o
        else:
            j = core - 4
            out[B + 2 * j:B + 2 * j + 2] = o
    return out
